# revision 11
# baseline (speedup 1.0000x reference)
"""Trainium2 Bass kernel for nn_Encoder_block (B=128,S=512,D=24,H=4,HD=6,DFF=48).

Data parallel over batch: 16 batches/core x 8 cores. Per core, T-layout
([d, token], d on partitions) with 4 batches banded per 128 partitions.

v2 speedups over the 478us baseline:
  - bf16 QKV/Wo matmuls and fp32r LN/FFN matmuls (1 cyc/row instead of 4).
  - softmax exp is a single Schraudolph-style affine per score pair: fp8e4m3
    BITS = round(s_raw * 8*log2e/sqrt(6) + 55.66) computed by one
    tensor_scalar/activation into a uint8-bitcast of the E tile. Replaces
    exact ACT exp + separate fp8 quantize.
  - AV uses fp8 DoubleRow matmuls whose two planes carry two HEADS via
    block-diagonal V weights: 8 matmuls x 512 cols x 0.5 cyc covers the whole
    attention-value product, landing directly in banded T-layout with the
    softmax denominators riding along as ones-columns.
  - softmax normalize: ACT copy UO->bf16, PE selector-broadcast of the
    denominator row, DVE reciprocal + multiply (no DMA broadcasts).
  - LayerNorm: selector matmuls for mean/var, PE broadcast of -mu and
    g*rstd (g folded into the selector weights), everything fp32r.
  - per-batch "tail" (normalize+Wo) emitted one batch late so PE stays busy.
"""

import os
import sys

import numpy as np

for _p in ("/opt/trn_rl_repo", "/opt/trn_rl_repo/concourse"):
    if os.path.isdir(_p) and _p not in sys.path:
        sys.path.insert(0, _p)

import concourse.bass as bass
import concourse.bacc as bacc
import concourse.mybir as mybir
import concourse.tile as tile
from concourse.bass_utils import run_bass_kernel_spmd

F32 = mybir.dt.float32
F32R = mybir.dt.float32r
BF16 = mybir.dt.bfloat16
FP8 = mybir.dt.float8e4
U8 = mybir.dt.uint8
AF = mybir.ActivationFunctionType
ALU = mybir.AluOpType
DR = mybir.MatmulPerfMode.DoubleRow

B, S, D = 128, 512, 24
H, HD, DFF = 4, 6, 48
EPS = 1e-5
NCORES = 8
NB = B // NCORES          # batches per core = 16
SCALE = 1.0 / np.sqrt(HD)
EA = float(8.0 * np.log2(np.e) * SCALE)   # fp8e4m3 bits slope
EB = 55.66                                # fp8e4m3 bits offset (calibrated)

# E-gen engine per (t, hp) slot: balance ACT vs DVE load
EGEN_ENGINES = ["act", "dve", "act", "dve", "act", "dve", "act", "act"]


def _host_consts(Wq, Wk, Wv, Wo, W1, W2, g1, b1, g2, b2):
    import ml_dtypes
    c = {}
    # QKV lhsT (bf16), split into fp8-DoubleRow planes: plane i holds head
    # dims {3i+jj}; wq0 also carries all of V at cols 8:32.
    def _plane(W, i, with_v=False):
        w = np.zeros((D, 128), np.float32)
        for h in range(H):
            for jj in range(3):
                w[:, 32 * h + jj] = W[6 * h + 3 * i + jj, :]
        if with_v:
            for dv in range(D):
                w[:, 8 + dv] = Wv[dv, :]
        out = np.zeros((128, 128), np.float32)
        for p in range(4):
            out[32 * p : 32 * p + D, :] = w
        return out.astype(ml_dtypes.bfloat16)

    c["wq0"] = _plane(Wq, 0, with_v=True)
    c["wq1"] = _plane(Wq, 1)
    c["wk0"] = _plane(Wk, 0)
    c["wk1"] = _plane(Wk, 1)

    # Wo lhsT bf16: rows 32h+1+j = Wo[:, 6h+j] (row 32h is the denominator)
    WOE = np.zeros((128, 32), np.float32)
    for h in range(H):
        for j in range(HD):
            WOE[32 * h + 1 + j, 0:D] = Wo[:, 6 * h + j]
    c["woe"] = WOE.astype(ml_dtypes.bfloat16)

    # LN selectors (fp32r): cb1 col 32p = -1/24 over band p; cb2 = +1/24
    CB1 = np.zeros((128, 128), np.float32)
    CB2 = np.zeros((128, 128), np.float32)
    for p in range(4):
        CB1[32 * p : 32 * p + D, 32 * p] = -1.0 / D
        CB2[32 * p : 32 * p + D, 32 * p] = 1.0 / D
    c["cb1"] = CB1
    c["cb2"] = CB2

    # broadcast selectors: col m -> 1 at row 32*(m//32); selg folds g
    SELR = np.zeros((128, 128), np.float32)
    SELG1 = np.zeros((128, 128), np.float32)
    SELG2 = np.zeros((128, 128), np.float32)
    for m in range(128):
        SELR[32 * (m // 32), m] = 1.0
        if m % 32 < D:
            SELG1[32 * (m // 32), m] = g1[m % 32]
            SELG2[32 * (m // 32), m] = g2[m % 32]
    c["selr"] = SELR
    c["selg1"] = SELG1
    c["selg2"] = SELG2
    c["selb"] = SELR.astype(ml_dtypes.bfloat16)

    # FFN W1 lhsT bf16: variant p: rows 32p+d, col 64p+m = W1[m, d]
    W1E = np.zeros((128, 4 * 64), np.float32)
    for p in range(4):
        W1E[32 * p : 32 * p + D, 64 * p : 64 * p + DFF] = W1.T
    c["w1e"] = W1E.astype(ml_dtypes.bfloat16)

    # FFN W2 lhsT bf16: even variant rows 0:48, odd rows 64:112
    W2E = np.zeros((128, 2 * 32), np.float32)
    W2E[0:DFF, 0:D] = W2.T
    W2E[64 : 64 + DFF, 32 : 32 + D] = W2.T
    c["w2e"] = W2E.astype(ml_dtypes.bfloat16)

    c["idtb"] = np.eye(32, dtype=ml_dtypes.float8_e4m3)

    # banded biases (only used when nonzero)
    GB = np.zeros((128, 2), np.float32)
    for p in range(4):
        GB[32 * p : 32 * p + D, 0] = b1
        GB[32 * p : 32 * p + D, 1] = b2
    c["gb"] = GB
    return c


CONST_SPECS = {
    "wq0": ((128, 128), BF16),
    "wq1": ((128, 128), BF16),
    "wk0": ((128, 128), BF16),
    "wk1": ((128, 128), BF16),
    "woe": ((128, 32), BF16),
    "cb1": ((128, 128), F32R),
    "cb2": ((128, 128), F32R),
    "selr": ((128, 128), F32R),
    "selg1": ((128, 128), F32R),
    "selg2": ((128, 128), F32R),
    "selb": ((128, 128), BF16),
    "w1e": ((128, 4 * 64), BF16),
    "w2e": ((128, 2 * 32), BF16),
    "idtb": ((32, 32), FP8),
    "gb": ((128, 2), F32),
}


def _pin_act_tables():
    """Pin Exp/Ln to natural_log_exp_and_others so LN's Ln+Exp never thrash."""
    import concourse.bacc as _bacc
    if getattr(_bacc, "_act_tables_pinned", False):
        return
    _orig = _bacc.get_activation_tables

    def _patched(arch):
        tables = dict(_orig(arch))
        keep = "natural_log_exp_and_others"
        for name in list(tables):
            if name != keep and (AF.Exp in tables[name] or AF.Ln in tables[name]):
                tables[name] = set()
        return tables

    _bacc.get_activation_tables = _patched
    _bacc._act_tables_pinned = True


def build_nc(nb: int = NB, use_b: bool = False) -> bass.Bass:
    _pin_act_tables()
    ngroups = nb // 4
    nc = bacc.Bacc()
    x_in = nc.dram_tensor("x", [nb, S, D], F32, kind="ExternalInput")
    out = nc.dram_tensor("out", [nb, S, D], F32, kind="ExternalOutput")
    cin = {
        k: nc.dram_tensor(k, list(sh), dt, kind="ExternalInput")
        for k, (sh, dt) in CONST_SPECS.items()
    }

    with tile.TileContext(nc) as tc:
        import contextlib

        ctx = contextlib.ExitStack()
        with ctx:
            constp = ctx.enter_context(tc.tile_pool(name="consts", bufs=1))
            vdp = ctx.enter_context(tc.tile_pool(name="vdp", bufs=1))
            xnp = ctx.enter_context(tc.tile_pool(name="xn", bufs=2))
            xtp = ctx.enter_context(tc.tile_pool(name="xt", bufs=2))
            xtbp = ctx.enter_context(tc.tile_pool(name="xtb", bufs=2))
            qkp = ctx.enter_context(tc.tile_pool(name="qk", bufs=2))
            ep = ctx.enter_context(tc.tile_pool(name="e", bufs=2))
            uosp = ctx.enter_context(tc.tile_pool(name="uos", bufs=2))
            rcpp = ctx.enter_context(tc.tile_pool(name="rcp", bufs=2))
            otp = ctx.enter_context(tc.tile_pool(name="ot", bufs=2))
            y1p = ctx.enter_context(tc.tile_pool(name="y1", bufs=2))
            lnp = ctx.enter_context(tc.tile_pool(name="ln", bufs=2))
            x1p = ctx.enter_context(tc.tile_pool(name="x1", bufs=2))
            hsp = ctx.enter_context(tc.tile_pool(name="hs", bufs=2))
            fsp = ctx.enter_context(tc.tile_pool(name="fs", bufs=2))
            ytp = ctx.enter_context(tc.tile_pool(name="yt", bufs=2))
            # PSUM: stt 2x[128,1024] + half 2x[128,512] + uo 2x[128,512] = 8 banks
            psA = ctx.enter_context(tc.tile_pool(name="psA", bufs=2, space="PSUM"))
            psB = ctx.enter_context(tc.tile_pool(name="psB", bufs=2, space="PSUM"))
            psC = ctx.enter_context(tc.tile_pool(name="psC", bufs=2, space="PSUM"))

            C = {}
            for k, (sh, dt) in CONST_SPECS.items():
                t = constp.tile(list(sh), dt, name=f"c_{k}")
                nc.sync.dma_start(out=t, in_=cin[k][:, :])
                C[k] = t
            eps_t = constp.tile([128, 1], F32, name="c_eps")
            nc.vector.memset(eps_t, EPS)

            # two persistent VD tiles [128, t4, hp2, i2, m128] fp8 (ping-pong).
            # ones preset at m = 32*(2hp+i) + {0, 7..31} inside each head band
            # (denominator column + finite padding rows for UO).
            vd_tiles = []
            for v in range(2):
                vt = vdp.tile([128, 4, 2, 2, 128], FP8, name=f"vd{v}")
                nc.gpsimd.memset(vt[:, :, :, :, :], 0.0)
                for hp in range(2):
                    for i in range(2):
                        m0 = 32 * (2 * hp + i)
                        nc.gpsimd.memset(vt[:, :, hp, i, m0 : m0 + 1], 1.0)
                        nc.gpsimd.memset(vt[:, :, hp, i, m0 + 7 : m0 + 32], 1.0)
                vd_tiles.append(vt)

            def ln_stages(Y, selg, bslice, OUT, outb=None):
                """LayerNorm over banded d of Y [128,512] fp32r, split into 4
                stages so PE never blocks on the DVE/ACT hops in between."""
                st = {}

                def s1():
                    mps = psB.tile([128, S], F32, name="mps", tag="half")
                    nc.tensor.matmul(mps[:, :], C["cb1"][:, :], Y[:, :],
                                     start=True, stop=True)
                    MU1 = lnp.tile([128, S], F32R, name="mu1", tag="mu")
                    nc.vector.tensor_copy(MU1[:, :], mps[:, :])
                    st["MU1"] = MU1

                def s2():
                    bcmu = psB.tile([128, S], F32, name="bcmu", tag="half")
                    nc.tensor.matmul(bcmu[:, :], C["selr"][:, :], st["MU1"][:, :],
                                     start=True, stop=True)
                    YC = lnp.tile([128, S], F32R, name="yc", tag="yc")
                    nc.vector.tensor_add(YC[:, :], Y[:, :], bcmu[:, :])
                    YC2 = lnp.tile([128, S], F32R, name="yc2", tag="yc2")
                    nc.gpsimd.tensor_mul(YC2[:, :], YC[:, :], YC[:, :])
                    st["YC"], st["YC2"] = YC, YC2

                def s3():
                    vps = psB.tile([128, S], F32, name="vps", tag="half")
                    nc.tensor.matmul(vps[:, :], C["cb2"][:, :], st["YC2"][:, :],
                                     start=True, stop=True)
                    LNV = lnp.tile([128, S], F32, name="lnv", tag="lnv")
                    nc.scalar.activation(LNV[:, :], vps[:, :], AF.Ln,
                                         bias=eps_t[:, :])
                    RS1 = lnp.tile([128, S], F32R, name="rs1", tag="rs")
                    nc.scalar.activation(RS1[:, :], LNV[:, :], AF.Exp, scale=-0.5)
                    st["RS1"] = RS1

                def s4():
                    bcrs = psB.tile([128, S], F32, name="bcrs", tag="half")
                    nc.tensor.matmul(bcrs[:, :], selg[:, :], st["RS1"][:, :],
                                     start=True, stop=True)
                    if use_b:
                        TMP = lnp.tile([128, S], F32R, name="lntmp", tag="tmp")
                        nc.vector.tensor_mul(TMP[:, :], st["YC"][:, :], bcrs[:, :])
                        nc.vector.tensor_scalar(out=OUT[:, :], in0=TMP[:, :],
                                                scalar1=bslice, scalar2=None,
                                                op0=ALU.add)
                    else:
                        nc.vector.tensor_mul(OUT[:, :], st["YC"][:, :], bcrs[:, :])
                    if outb is not None:
                        nc.gpsimd.tensor_copy(outb[:, :], OUT[:, :])

                return [s1, s2, s3, s4]

            def batch_head(g, p, XT4b):
                """QKV + fp8-DR scores + E-gen for batch 4g+p."""
                pss = []
                for name, w in (("psq0", "wq0"), ("psq1", "wq1"),
                                ("psk0", "wk0"), ("psk1", "wk1")):
                    pst = psB.tile([128, S], F32, name=name, tag="half")
                    nc.tensor.matmul(
                        pst[:, :], C[w][32 * p : 32 * p + D, :],
                        XT4b[32 * p : 32 * p + D, :],
                        start=True, stop=True, tile_position=(32 * p, 0),
                    )
                    pss.append(pst)
                QKq = qkp.tile([128, 2, S], FP8, name="qkq", tag="qkq")
                QKk = qkp.tile([128, 2, S], FP8, name="qkk", tag="qkk")
                nc.vector.tensor_copy(QKq[:, 0, :], pss[0][:, :])
                nc.scalar.activation(QKq[:, 1, :], pss[1][:, :], AF.Copy)
                nc.vector.tensor_copy(QKk[:, 0, :], pss[2][:, :])
                nc.scalar.activation(QKk[:, 1, :], pss[3][:, :], AF.Copy)

                # V^T via PE transposes of plane0 rows 0:32 (V at rows 8:32).
                # fp8 transpose requires output element step 2.
                psv = psB.tile([128, 4 * 64], FP8, name="psv", tag="half")
                for t in range(4):
                    pv_dst = bass.AP(
                        tensor=psv.tensor, offset=psv.offset + 64 * t,
                        ap=[list(psv.ap[0]), [2, 32]],
                    )
                    nc.tensor.transpose(
                        pv_dst,
                        QKq[0:32, 0, 128 * t : 128 * (t + 1)],
                        C["idtb"][:, :],
                    )
                VD = vd_tiles[(4 * g + p) % 2]
                # VD[k, t, hp, i, 32*(2hp+i)+1 : +7] = psv[k, 32t+8+6(2hp+i) : +6]
                vd_dst = bass.AP(
                    tensor=VD.tensor, offset=VD.offset + 1,
                    ap=[list(VD.ap[0]), [512, 4], [160, 4], [1, 6]],
                )
                vd_src = bass.AP(
                    tensor=psv.tensor, offset=psv.offset + 16,
                    ap=[list(psv.ap[0]), [64, 4], [12, 4], [2, 6]],
                )
                nc.vector.tensor_copy(vd_dst, vd_src)

                # scores (S^T, raw) via fp8 DoubleRow + E-gen (fp8 bits affine)
                E = ep.tile([128, 4, 2, 2, S], FP8, name="e")
                for t in range(4):
                    for hp in range(2):
                        stt = psA.tile([128, 2 * S], F32, name="stt", tag="big")
                        for i in range(2):
                            h = 2 * hp + i
                            nc.tensor.matmul(
                                stt[:, 512 * i : 512 * (i + 1)],
                                QKk[32 * h : 32 * h + 3, :,
                                    128 * t : 128 * (t + 1)],
                                QKq[32 * h : 32 * h + 3, :, 0:S],
                                start=True, stop=True,
                                perf_mode=DR,
                                tile_position=(32 * h, 0),
                                skip_group_check=True,
                            )
                        eng = EGEN_ENGINES[2 * t + hp]
                        e_out = E[:, t, hp, :, :].bitcast(U8)
                        s_in = stt.rearrange("p (i q) -> p i q", i=2)
                        if eng == "act":
                            nc.scalar.activation(e_out, s_in, AF.Copy,
                                                 bias=EB, scale=EA)
                        else:
                            nc.vector.tensor_scalar(
                                out=e_out, in0=s_in, scalar1=EA, scalar2=EB,
                                op0=ALU.mult, op1=ALU.add,
                            )

                return {"E": E, "VD": VD, "p": p}

            def batch_av(state):
                """fp8 DoubleRow AV, two heads per matmul via block-diag VD."""
                E, VD = state["E"], state["VD"]
                UO = psC.tile([128, S], F32, name="uo", tag="uo")
                idx = 0
                for t in range(4):
                    for hp in range(2):
                        nc.tensor.matmul(
                            UO[:, :], VD[:, t, hp, :, :], E[:, t, hp, :, :],
                            start=(idx == 0), stop=(idx == 7),
                            perf_mode=DR, tile_position=(0, 0),
                        )
                        idx += 1
                state["UO"] = UO

            def batch_tail(state, XT4, Y1):
                """normalize + Wo + residual for a batch (lagged emission)."""
                UO = state["UO"]
                p = state["p"]
                UOs = uosp.tile([128, S], BF16, name="uos")
                nc.scalar.activation(UOs[:, :], UO[:, :], AF.Copy)
                denps = psB.tile([128, S], F32, name="denps", tag="half")
                nc.tensor.matmul(denps[:, :], C["selb"][:, :], UOs[:, :],
                                 start=True, stop=True)
                RCP = rcpp.tile([128, S], F32, name="rcpt")
                nc.vector.reciprocal_approx_fast(RCP[:, :], denps[:, :])
                OTn = otp.tile([128, S], BF16, name="otn")
                nc.gpsimd.tensor_mul(OTn[:, :], UOs[:, :], RCP[:, :])
                wops = psB.tile([32, S], F32, name="wops", tag="half")
                nc.tensor.matmul(wops[:, :], C["woe"][:, :], OTn[:, :],
                                 start=True, stop=True)
                nc.vector.tensor_add(
                    Y1[32 * p : 32 * p + 32, :], wops[:, :],
                    XT4[32 * p : 32 * p + 32, :],
                )

            def group_stages(g, Y1):
                """LN1 + FFN + LN2 + output store for group g as a stage list."""
                X1 = x1p.tile([128, S], F32R, name="x1")
                X1b = x1p.tile([128, S], BF16, name="x1b", tag="x1b")
                stages = ln_stages(Y1, C["selg1"], C["gb"][:, 0:1], X1, outb=X1b)
                st = {}

                def ffn_a():
                    F4 = psC.tile([128, S], F32, name="f4", tag="uo")
                    hps = psB.tile([128, S], F32, name="hps", tag="half")
                    for j in range(2):
                        nc.tensor.matmul(
                            hps[64 * j : 64 * j + 64, :],
                            C["w1e"][:, 64 * j : 64 * (j + 1)],
                            X1b[:, :],
                            start=True, stop=True, tile_position=(0, 64 * j),
                            skip_group_check=True,
                        )
                    HS = hsp.tile([128, S], BF16, name="hs")
                    nc.scalar.activation(HS[:, :], hps[:, :], AF.Relu)
                    st["F4"], st["HS"] = F4, HS

                def ffn_b():
                    F4 = st["F4"]
                    for j in range(2):
                        nc.tensor.matmul(
                            F4[32 * j : 32 * j + 32, :],
                            C["w2e"][:, 32 * j : 32 * (j + 1)],
                            st["HS"][:, :],
                            start=True, stop=True, tile_position=(0, 32 * j),
                            skip_group_check=True,
                        )
                    hps = psB.tile([128, S], F32, name="hps2", tag="half")
                    for j in range(2):
                        nc.tensor.matmul(
                            hps[64 * j : 64 * j + 64, :],
                            C["w1e"][:, 64 * (2 + j) : 64 * (3 + j)],
                            X1b[:, :],
                            start=True, stop=True, tile_position=(0, 64 * j),
                            skip_group_check=True,
                        )
                    HS2 = hsp.tile([128, S], BF16, name="hs2", tag="hs2")
                    nc.scalar.activation(HS2[:, :], hps[:, :], AF.Relu)
                    st["HS2"] = HS2

                def ffn_c():
                    F4 = st["F4"]
                    for j in range(2):
                        nc.tensor.matmul(
                            F4[32 * (2 + j) : 32 * (3 + j), :],
                            C["w2e"][:, 32 * j : 32 * (j + 1)],
                            st["HS2"][:, :],
                            start=True, stop=True, tile_position=(0, 32 * (2 + j)),
                            skip_group_check=True,
                        )
                    FS0 = fsp.tile([128, S], F32R, name="fs0", tag="fs0")
                    nc.scalar.activation(FS0[:, :], F4[:, :], AF.Relu)
                    FS = fsp.tile([128, S], F32R, name="fst", tag="fst")
                    nc.gpsimd.tensor_add(FS[:, :], FS0[:, :], X1[:, :])
                    st["FS"] = FS

                stages += [ffn_a, ffn_b, ffn_c]

                Y2N = ytp.tile([128, S], F32, name="y2n", tag="y2n")
                ln2 = [None]

                def ln2_s1():
                    ln2[0] = ln_stages(st["FS"], C["selg2"], C["gb"][:, 1:2], Y2N)
                    ln2[0][0]()

                def emit_out():
                    Y2T = ytp.tile([128, S], F32, name="y2t", tag="y2t")
                    nc.vector.transpose(Y2T[:, :], Y2N[:, :])
                    for p in range(4):
                        b = 4 * g + p
                        nc.sync.dma_start(
                            out=out[b].rearrange("(f r) d -> r f d", r=32),
                            in_=Y2T[32 * p : 32 * p + 32, :].rearrange(
                                "r (f c) -> r f c", c=32
                            )[:, :, 0:D],
                        )

                stages += [ln2_s1,
                           lambda: ln2[0][1](),
                           lambda: ln2[0][2](),
                           lambda: ln2[0][3](),
                           emit_out]
                return stages

            # slot machine: headA(i) | AV(i-1) | tail(i-2) | <=3 group stages
            states = {}
            gctx = {}
            pending = []
            nslots = 4 * ngroups

            def emit_slot(i):
                g, p = divmod(i, 4)
                if p == 0:
                    U = xnp.tile([128, 16, 32], F32, name="xu")
                    nc.gpsimd.memset(U[:, :, D:32], 0.0)
                    for pp in range(4):
                        nc.sync.dma_start(
                            out=U[32 * pp : 32 * pp + 32, :, 0:D],
                            in_=x_in[4 * g + pp].rearrange("(f c) d -> c f d", c=32),
                        )
                    XT4 = xtp.tile([128, S], F32, name="xt4")
                    nc.vector.transpose(XT4[:, :], U.rearrange("p a b -> p (a b)"))
                    XT4b = xtbp.tile([128, S], BF16, name="xt4b")
                    nc.gpsimd.tensor_copy(XT4b[:, :], XT4[:, :])
                    Y1 = y1p.tile([128, S], F32R, name="y1")
                    gctx[g] = (XT4, XT4b, Y1)
                XT4, XT4b, Y1 = gctx[g]
                states[i] = batch_head(g, p, XT4b)
                if i - 1 in states:
                    batch_av(states[i - 1])
                if i - 2 in states:
                    s = states.pop(i - 2)
                    g2 = (i - 2) // 4
                    batch_tail(s, gctx[g2][0], gctx[g2][2])
                    if (i - 2) % 4 == 3:
                        pending.extend(group_stages(g2, gctx[g2][2]))
                for _ in range(3):
                    if pending:
                        pending.pop(0)()

            for i in range(nslots):
                emit_slot(i)
            # drain
            batch_av(states[nslots - 1])
            for i in (nslots - 2, nslots - 1):
                s = states.pop(i)
                g2 = i // 4
                batch_tail(s, gctx[g2][0], gctx[g2][2])
                if i % 4 == 3:
                    pending.extend(group_stages(g2, gctx[g2][2]))
            while pending:
                pending.pop(0)()

    nc.compile()
    return nc


def build_nc(nb: int = NB, use_b: bool = False) -> bass.Bass:
    _pin_act_tables()
    ngroups = nb // 4
    nc = bacc.Bacc()
    x_in = nc.dram_tensor("x", [nb, S, D], F32, kind="ExternalInput")
    out = nc.dram_tensor("out", [nb, S, D], F32, kind="ExternalOutput")
    cin = {
        k: nc.dram_tensor(k, list(sh), dt, kind="ExternalInput")
        for k, (sh, dt) in CONST_SPECS.items()
    }

    with tile.TileContext(nc) as tc:
        import contextlib

        ctx = contextlib.ExitStack()
        with ctx:
            constp = ctx.enter_context(tc.tile_pool(name="consts", bufs=1))
            vdp = ctx.enter_context(tc.tile_pool(name="vdp", bufs=1))
            xnp = ctx.enter_context(tc.tile_pool(name="xn", bufs=2))
            xtp = ctx.enter_context(tc.tile_pool(name="xt", bufs=2))
            xtbp = ctx.enter_context(tc.tile_pool(name="xtb", bufs=2))
            qkp = ctx.enter_context(tc.tile_pool(name="qk", bufs=2))
            ep = ctx.enter_context(tc.tile_pool(name="e", bufs=2))
            uosp = ctx.enter_context(tc.tile_pool(name="uos", bufs=2))
            rcpp = ctx.enter_context(tc.tile_pool(name="rcp", bufs=2))
            otp = ctx.enter_context(tc.tile_pool(name="ot", bufs=2))
            y1p = ctx.enter_context(tc.tile_pool(name="y1", bufs=2))
            lnp = ctx.enter_context(tc.tile_pool(name="ln", bufs=2))
            x1p = ctx.enter_context(tc.tile_pool(name="x1", bufs=2))
            hsp = ctx.enter_context(tc.tile_pool(name="hs", bufs=2))
            fsp = ctx.enter_context(tc.tile_pool(name="fs", bufs=2))
            ytp = ctx.enter_context(tc.tile_pool(name="yt", bufs=2))
            # PSUM: stt 2x[128,1024] + half 2x[128,512] + uo 2x[128,512] = 8 banks
            psA = ctx.enter_context(tc.tile_pool(name="psA", bufs=2, space="PSUM"))
            psB = ctx.enter_context(tc.tile_pool(name="psB", bufs=2, space="PSUM"))
            psC = ctx.enter_context(tc.tile_pool(name="psC", bufs=2, space="PSUM"))

            C = {}
            for k, (sh, dt) in CONST_SPECS.items():
                t = constp.tile(list(sh), dt, name=f"c_{k}")
                nc.sync.dma_start(out=t, in_=cin[k][:, :])
                C[k] = t
            eps_t = constp.tile([128, 1], F32, name="c_eps")
            nc.vector.memset(eps_t, EPS)

            # two persistent VD tiles [128, t4, hp2, i2, m128] fp8 (ping-pong).
            # ones preset at m = 32*(2hp+i) + {0, 7..31} inside each head band
            # (denominator column + finite padding rows for UO).
            vd_tiles = []
            for v in range(2):
                vt = vdp.tile([128, 4, 2, 2, 128], FP8, name=f"vd{v}")
                nc.gpsimd.memset(vt[:, :, :, :, :], 0.0)
                for hp in range(2):
                    for i in range(2):
                        m0 = 32 * (2 * hp + i)
                        nc.gpsimd.memset(vt[:, :, hp, i, m0 : m0 + 1], 1.0)
                        nc.gpsimd.memset(vt[:, :, hp, i, m0 + 7 : m0 + 32], 1.0)
                vd_tiles.append(vt)

            def ln_stages(Y, selg, bslice, OUT, outb=None):
                """LayerNorm over banded d of Y [128,512] fp32r, split into 4
                stages so PE never blocks on the DVE/ACT hops in between."""
                st = {}

                def s1():
                    mps = psB.tile([128, S], F32, name="mps", tag="half")
                    nc.tensor.matmul(mps[:, :], C["cb1"][:, :], Y[:, :],
                                     start=True, stop=True)
                    MU1 = lnp.tile([128, S], F32R, name="mu1", tag="mu")
                    nc.vector.tensor_copy(MU1[:, :], mps[:, :])
                    st["MU1"] = MU1

                def s2():
                    bcmu = psB.tile([128, S], F32, name="bcmu", tag="half")
                    nc.tensor.matmul(bcmu[:, :], C["selr"][:, :], st["MU1"][:, :],
                                     start=True, stop=True)
                    YC = lnp.tile([128, S], F32R, name="yc", tag="yc")
                    nc.vector.tensor_add(YC[:, :], Y[:, :], bcmu[:, :])
                    YC2 = lnp.tile([128, S], F32R, name="yc2", tag="yc2")
                    nc.gpsimd.tensor_mul(YC2[:, :], YC[:, :], YC[:, :])
                    st["YC"], st["YC2"] = YC, YC2

                def s3():
                    vps = psB.tile([128, S], F32, name="vps", tag="half")
                    nc.tensor.matmul(vps[:, :], C["cb2"][:, :], st["YC2"][:, :],
                                     start=True, stop=True)
                    LNV = lnp.tile([128, S], F32, name="lnv", tag="lnv")
                    nc.scalar.activation(LNV[:, :], vps[:, :], AF.Ln,
                                         bias=eps_t[:, :])
                    RS1 = lnp.tile([128, S], F32R, name="rs1", tag="rs")
                    nc.scalar.activation(RS1[:, :], LNV[:, :], AF.Exp, scale=-0.5)
                    st["RS1"] = RS1

                def s4():
                    bcrs = psB.tile([128, S], F32, name="bcrs", tag="half")
                    nc.tensor.matmul(bcrs[:, :], selg[:, :], st["RS1"][:, :],
                                     start=True, stop=True)
                    if use_b:
                        TMP = lnp.tile([128, S], F32R, name="lntmp", tag="tmp")
                        nc.vector.tensor_mul(TMP[:, :], st["YC"][:, :], bcrs[:, :])
                        nc.vector.tensor_scalar(out=OUT[:, :], in0=TMP[:, :],
                                                scalar1=bslice, scalar2=None,
                                                op0=ALU.add)
                    else:
                        nc.vector.tensor_mul(OUT[:, :], st["YC"][:, :], bcrs[:, :])
                    if outb is not None:
                        nc.gpsimd.tensor_copy(outb[:, :], OUT[:, :])

                return [s1, s2, s3, s4]

            def batch_head(g, p, XT4b):
                """QKV + fp8-DR scores + E-gen for batch 4g+p."""
                pss = []
                for name, w in (("psq0", "wq0"), ("psq1", "wq1"),
                                ("psk0", "wk0"), ("psk1", "wk1")):
                    pst = psB.tile([128, S], F32, name=name, tag="half")
                    nc.tensor.matmul(
                        pst[:, :], C[w][32 * p : 32 * p + D, :],
                        XT4b[32 * p : 32 * p + D, :],
                        start=True, stop=True, tile_position=(32 * p, 0),
                    )
                    pss.append(pst)
                QKq = qkp.tile([128, 2, S], FP8, name="qkq", tag="qkq")
                QKk = qkp.tile([128, 2, S], FP8, name="qkk", tag="qkk")
                nc.vector.tensor_copy(QKq[:, 0, :], pss[0][:, :])
                nc.scalar.activation(QKq[:, 1, :], pss[1][:, :], AF.Copy)
                nc.vector.tensor_copy(QKk[:, 0, :], pss[2][:, :])
                nc.scalar.activation(QKk[:, 1, :], pss[3][:, :], AF.Copy)

                # V^T via PE transposes of plane0 rows 0:32 (V at rows 8:32).
                # fp8 transpose requires output element step 2.
                psv = psB.tile([128, 4 * 64], FP8, name="psv", tag="half")
                for t in range(4):
                    pv_dst = bass.AP(
                        tensor=psv.tensor, offset=psv.offset + 64 * t,
                        ap=[list(psv.ap[0]), [2, 32]],
                    )
                    nc.tensor.transpose(
                        pv_dst,
                        QKq[0:32, 0, 128 * t : 128 * (t + 1)],
                        C["idtb"][:, :],
                    )
                VD = vd_tiles[(4 * g + p) % 2]
                # VD[k, t, hp, i, 32*(2hp+i)+1 : +7] = psv[k, 32t+8+6(2hp+i) : +6]
                vd_dst = bass.AP(
                    tensor=VD.tensor, offset=VD.offset + 1,
                    ap=[list(VD.ap[0]), [512, 4], [160, 4], [1, 6]],
                )
                vd_src = bass.AP(
                    tensor=psv.tensor, offset=psv.offset + 16,
                    ap=[list(psv.ap[0]), [64, 4], [12, 4], [2, 6]],
                )
                nc.vector.tensor_copy(vd_dst, vd_src)

                # scores (S^T, raw) via fp8 DoubleRow + E-gen (fp8 bits affine)
                E = ep.tile([128, 4, 2, 2, S], FP8, name="e")
                for t in range(4):
                    for hp in range(2):
                        stt = psA.tile([128, 2 * S], F32, name="stt", tag="big")
                        for i in range(2):
                            h = 2 * hp + i
                            nc.tensor.matmul(
                                stt[:, 512 * i : 512 * (i + 1)],
                                QKk[32 * h : 32 * h + 3, :,
                                    128 * t : 128 * (t + 1)],
                                QKq[32 * h : 32 * h + 3, :, 0:S],
                                start=True, stop=True,
                                perf_mode=DR,
                                tile_position=(32 * h, 0),
                                skip_group_check=True,
                            )
                        eng = EGEN_ENGINES[2 * t + hp]
                        e_out = E[:, t, hp, :, :].bitcast(U8)
                        s_in = stt.rearrange("p (i q) -> p i q", i=2)
                        if eng == "act":
                            nc.scalar.activation(e_out, s_in, AF.Copy,
                                                 bias=EB, scale=EA)
                        else:
                            nc.vector.tensor_scalar(
                                out=e_out, in0=s_in, scalar1=EA, scalar2=EB,
                                op0=ALU.mult, op1=ALU.add,
                            )

                return {"E": E, "VD": VD, "p": p}

            def batch_av(state):
                """fp8 DoubleRow AV, two heads per matmul via block-diag VD."""
                E, VD = state["E"], state["VD"]
                UO = psC.tile([128, S], F32, name="uo", tag="uo")
                idx = 0
                for t in range(4):
                    for hp in range(2):
                        nc.tensor.matmul(
                            UO[:, :], VD[:, t, hp, :, :], E[:, t, hp, :, :],
                            start=(idx == 0), stop=(idx == 7),
                            perf_mode=DR, tile_position=(0, 0),
                        )
                        idx += 1
                state["UO"] = UO

            def batch_tail(state, XT4, Y1):
                """normalize + Wo + residual for a batch (lagged emission)."""
                UO = state["UO"]
                p = state["p"]
                UOs = uosp.tile([128, S], BF16, name="uos")
                nc.scalar.activation(UOs[:, :], UO[:, :], AF.Copy)
                denps = psB.tile([128, S], F32, name="denps", tag="half")
                nc.tensor.matmul(denps[:, :], C["selb"][:, :], UOs[:, :],
                                 start=True, stop=True)
                RCP = rcpp.tile([128, S], F32, name="rcpt")
                nc.vector.reciprocal_approx_fast(RCP[:, :], denps[:, :])
                OTn = otp.tile([128, S], BF16, name="otn")
                nc.gpsimd.tensor_mul(OTn[:, :], UOs[:, :], RCP[:, :])
                wops = psB.tile([32, S], F32, name="wops", tag="half")
                nc.tensor.matmul(wops[:, :], C["woe"][:, :], OTn[:, :],
                                 start=True, stop=True)
                nc.vector.tensor_add(
                    Y1[32 * p : 32 * p + 32, :], wops[:, :],
                    XT4[32 * p : 32 * p + 32, :],
                )

            def group_stages(g, Y1):
                """LN1 + FFN + LN2 + output store for group g as a stage list."""
                X1 = x1p.tile([128, S], F32R, name="x1")
                X1b = x1p.tile([128, S], BF16, name="x1b", tag="x1b")
                stages = ln_stages(Y1, C["selg1"], C["gb"][:, 0:1], X1, outb=X1b)
                st = {}

                def ffn_a():
                    F4 = psC.tile([128, S], F32, name="f4", tag="uo")
                    hps = psB.tile([128, S], F32, name="hps", tag="half")
                    for j in range(2):
                        nc.tensor.matmul(
                            hps[64 * j : 64 * j + 64, :],
                            C["w1e"][:, 64 * j : 64 * (j + 1)],
                            X1b[:, :],
                            start=True, stop=True, tile_position=(0, 64 * j),
                            skip_group_check=True,
                        )
                    HS = hsp.tile([128, S], BF16, name="hs")
                    nc.scalar.activation(HS[:, :], hps[:, :], AF.Relu)
                    st["F4"], st["HS"] = F4, HS

                def ffn_b():
                    F4 = st["F4"]
                    for j in range(2):
                        nc.tensor.matmul(
                            F4[32 * j : 32 * j + 32, :],
                            C["w2e"][:, 32 * j : 32 * (j + 1)],
                            st["HS"][:, :],
                            start=True, stop=True, tile_position=(0, 32 * j),
                            skip_group_check=True,
                        )
                    hps = psB.tile([128, S], F32, name="hps2", tag="half")
                    for j in range(2):
                        nc.tensor.matmul(
                            hps[64 * j : 64 * j + 64, :],
                            C["w1e"][:, 64 * (2 + j) : 64 * (3 + j)],
                            X1b[:, :],
                            start=True, stop=True, tile_position=(0, 64 * j),
                            skip_group_check=True,
                        )
                    HS2 = hsp.tile([128, S], BF16, name="hs2", tag="hs2")
                    nc.scalar.activation(HS2[:, :], hps[:, :], AF.Relu)
                    st["HS2"] = HS2

                def ffn_c():
                    F4 = st["F4"]
                    for j in range(2):
                        nc.tensor.matmul(
                            F4[32 * (2 + j) : 32 * (3 + j), :],
                            C["w2e"][:, 32 * j : 32 * (j + 1)],
                            st["HS2"][:, :],
                            start=True, stop=True, tile_position=(0, 32 * (2 + j)),
                            skip_group_check=True,
                        )
                    FS0 = fsp.tile([128, S], F32R, name="fs0", tag="fs0")
                    nc.scalar.activation(FS0[:, :], F4[:, :], AF.Relu)
                    FS = fsp.tile([128, S], F32R, name="fst", tag="fst")
                    nc.gpsimd.tensor_add(FS[:, :], FS0[:, :], X1[:, :])
                    st["FS"] = FS

                stages += [ffn_a, ffn_b, ffn_c]

                Y2N = ytp.tile([128, S], F32, name="y2n", tag="y2n")
                ln2 = [None]

                def ln2_s1():
                    ln2[0] = ln_stages(st["FS"], C["selg2"], C["gb"][:, 1:2], Y2N)
                    ln2[0][0]()

                def emit_out():
                    Y2T = ytp.tile([128, S], F32, name="y2t", tag="y2t")
                    nc.vector.transpose(Y2T[:, :], Y2N[:, :])
                    for p in range(4):
                        b = 4 * g + p
                        nc.sync.dma_start(
                            out=out[b].rearrange("(f r) d -> r f d", r=32),
                            in_=Y2T[32 * p : 32 * p + 32, :].rearrange(
                                "r (f c) -> r f c", c=32
                            )[:, :, 0:D],
                        )

                stages += [ln2_s1,
                           lambda: ln2[0][1](),
                           lambda: ln2[0][2](),
                           lambda: ln2[0][3](),
                           emit_out]
                return stages

            # slot machine: headA(i) | AV(i-1) | tail(i-2) | <=3 group stages
            states = {}
            gctx = {}
            pending = []
            nslots = 4 * ngroups

            def emit_slot(i):
                g, p = divmod(i, 4)
                if p == 0:
                    U = xnp.tile([128, 16, 32], F32, name="xu")
                    nc.gpsimd.memset(U[:, :, D:32], 0.0)
                    for pp in range(4):
                        nc.sync.dma_start(
                            out=U[32 * pp : 32 * pp + 32, :, 0:D],
                            in_=x_in[4 * g + pp].rearrange("(f c) d -> c f d", c=32),
                        )
                    XT4 = xtp.tile([128, S], F32, name="xt4")
                    nc.vector.transpose(XT4[:, :], U.rearrange("p a b -> p (a b)"))
                    XT4b = xtbp.tile([128, S], BF16, name="xt4b")
                    nc.gpsimd.tensor_copy(XT4b[:, :], XT4[:, :])
                    Y1 = y1p.tile([128, S], F32R, name="y1")
                    gctx[g] = (XT4, XT4b, Y1)
                XT4, XT4b, Y1 = gctx[g]
                states[i] = batch_head(g, p, XT4b)
                if i - 1 in states:
                    batch_av(states[i - 1])
                if i - 2 in states:
                    s = states.pop(i - 2)
                    g2 = (i - 2) // 4
                    batch_tail(s, gctx[g2][0], gctx[g2][2])
                    if (i - 2) % 4 == 3:
                        pending.extend(group_stages(g2, gctx[g2][2]))
                for _ in range(3):
                    if pending:
                        pending.pop(0)()

            for i in range(nslots):
                emit_slot(i)
            # drain
            batch_av(states[nslots - 1])
            for i in (nslots - 2, nslots - 1):
                s = states.pop(i)
                g2 = i // 4
                batch_tail(s, gctx[g2][0], gctx[g2][2])
                if i % 4 == 3:
                    pending.extend(group_stages(g2, gctx[g2][2]))
            while pending:
                pending.pop(0)()

    nc.compile()
    return nc


def _enable_ldw_opt():
    """Flip walrus --enable-ldw-opt to true (hides LDWEIGHTS behind matmuls)."""
    import concourse.bass_utils as _bu
    if getattr(_bu, "_ldw_opt_patched", False):
        return
    _orig = _bu.run_command

    def _patched(cmd, *a, **kw):
        if isinstance(cmd, list):
            cmd = ["--enable-ldw-opt=true" if c == "--enable-ldw-opt=false" else c
                   for c in cmd]
        return _orig(cmd, *a, **kw)

    _bu.run_command = _patched
    _bu._ldw_opt_patched = True


_NC_CACHE: dict[tuple, bass.Bass] = {}


def _get_nc(nb: int, use_b: bool = False) -> bass.Bass:
    key = (nb, use_b)
    if key not in _NC_CACHE:
        _NC_CACHE[key] = build_nc(nb, use_b)
    return _NC_CACHE[key]


def kernel(x, Wq, Wk, Wv, Wo, W1, W2, g1, b1, g2, b2):
    x = np.asarray(x, np.float32)
    args = [np.asarray(a, np.float32) for a in (Wq, Wk, Wv, Wo, W1, W2, g1, b1, g2, b2)]
    consts = _host_consts(*args)
    use_b = bool(np.any(args[7]) or np.any(args[9]))
    nc = _get_nc(NB, use_b)
    in_maps = []
    for c in range(NCORES):
        m = {"x": np.ascontiguousarray(x[c * NB : (c + 1) * NB])}
        m.update(consts)
        in_maps.append(m)
    res = run_bass_kernel_spmd(nc, in_maps, list(range(NCORES)))
    return np.concatenate([r["out"] for r in res.results], axis=0)


# revision 22
# speedup vs baseline: 1.2323x; 1.2323x over previous
"""Trainium2 Bass kernel for nn_Encoder_block (B=128,S=512,D=24,H=4,HD=6,DFF=48).

Data parallel over batch: 16 batches/core x 8 cores. Per core, T-layout
([d, token], d on partitions) with 4 batches banded per 128 partitions.

v2 speedups over the 478us baseline:
  - bf16 QKV/Wo matmuls and fp32r LN/FFN matmuls (1 cyc/row instead of 4).
  - softmax exp is a single Schraudolph-style affine per score pair: fp8e4m3
    BITS = round(s_raw * 8*log2e/sqrt(6) + 55.66) computed by one
    tensor_scalar/activation into a uint8-bitcast of the E tile. Replaces
    exact ACT exp + separate fp8 quantize.
  - AV uses fp8 DoubleRow matmuls whose two planes carry two HEADS via
    block-diagonal V weights: 8 matmuls x 512 cols x 0.5 cyc covers the whole
    attention-value product, landing directly in banded T-layout with the
    softmax denominators riding along as ones-columns.
  - softmax normalize: ACT copy UO->bf16, PE selector-broadcast of the
    denominator row, DVE reciprocal + multiply (no DMA broadcasts).
  - LayerNorm: selector matmuls for mean/var, PE broadcast of -mu and
    g*rstd (g folded into the selector weights), everything fp32r.
  - per-batch "tail" (normalize+Wo) emitted one batch late so PE stays busy.
"""

import os
import sys

import numpy as np

for _p in ("/opt/trn_rl_repo", "/opt/trn_rl_repo/concourse"):
    if os.path.isdir(_p) and _p not in sys.path:
        sys.path.insert(0, _p)

import concourse.bass as bass
import concourse.bacc as bacc
import concourse.mybir as mybir
import concourse.tile as tile
from concourse.bass_utils import run_bass_kernel_spmd

F32 = mybir.dt.float32
F32R = mybir.dt.float32r
BF16 = mybir.dt.bfloat16
FP8 = mybir.dt.float8e4
U8 = mybir.dt.uint8
AF = mybir.ActivationFunctionType
ALU = mybir.AluOpType
DR = mybir.MatmulPerfMode.DoubleRow

B, S, D = 128, 512, 24
H, HD, DFF = 4, 6, 48
EPS = 1e-5
NCORES = 8
NB = B // NCORES          # batches per core = 16
SCALE = 1.0 / np.sqrt(HD)
EA = float(8.0 * np.log2(np.e) * SCALE)   # fp8e4m3 bits slope
EB = 55.66                                # fp8e4m3 bits offset (calibrated)

# E-gen engine per (t, hp) slot: balance ACT vs DVE load
EGEN_ENGINES = ["act", "dve", "act", "dve", "act", "dve", "act", "act"]


def _host_consts(Wq, Wk, Wv, Wo, W1, W2, g1, b1, g2, b2):
    import ml_dtypes
    c = {}
    # QKV lhsT (bf16): per band p: col 32h+j = Wq[6h+j,:], cols 8:32 = Wv rows
    wqk1 = np.zeros((D, 128), np.float32)
    wk2 = np.zeros((D, 128), np.float32)
    for h in range(H):
        for j in range(HD):
            wqk1[:, 32 * h + j] = Wq[6 * h + j, :]
            wk2[:, 32 * h + j] = Wk[6 * h + j, :]
    for dv in range(D):
        wqk1[:, 8 + dv] = Wv[dv, :]
    WQK1 = np.zeros((128, 128), np.float32)
    WK2 = np.zeros((128, 128), np.float32)
    for p in range(4):
        WQK1[32 * p : 32 * p + D, :] = wqk1
        WK2[32 * p : 32 * p + D, :] = wk2
    c["wqk1"] = WQK1.astype(ml_dtypes.bfloat16)
    c["wk2"] = WK2.astype(ml_dtypes.bfloat16)

    # Wo lhsT bf16: rows 32h+1+j = Wo[:, 6h+j] (row 32h is the denominator)
    WOE = np.zeros((128, 32), np.float32)
    for h in range(H):
        for j in range(HD):
            WOE[32 * h + 1 + j, 0:D] = Wo[:, 6 * h + j]
    c["woe"] = WOE.astype(ml_dtypes.bfloat16)

    # LN selectors (fp32r): cb1 col 32p = -1/24 over band p; cb2 = +1/24
    CB1 = np.zeros((128, 128), np.float32)
    CB2 = np.zeros((128, 128), np.float32)
    for p in range(4):
        CB1[32 * p : 32 * p + D, 32 * p] = -1.0 / D
        CB2[32 * p : 32 * p + D, 32 * p] = 1.0 / D
    c["cb1"] = CB1.astype(ml_dtypes.bfloat16)
    c["cb2"] = CB2.astype(ml_dtypes.bfloat16)

    # broadcast selectors: col m -> 1 at row 32*(m//32); selg folds g
    SELR = np.zeros((128, 128), np.float32)
    SELG1 = np.zeros((128, 128), np.float32)
    SELG2 = np.zeros((128, 128), np.float32)
    for m in range(128):
        SELR[32 * (m // 32), m] = 1.0
        if m % 32 < D:
            SELG1[32 * (m // 32), m] = g1[m % 32]
            SELG2[32 * (m // 32), m] = g2[m % 32]
    c["selr"] = SELR.astype(ml_dtypes.bfloat16)
    c["selg1"] = SELG1.astype(ml_dtypes.bfloat16)
    c["selg2"] = SELG2.astype(ml_dtypes.bfloat16)
    c["selb"] = SELR.astype(ml_dtypes.bfloat16)

    # FFN W1 lhsT bf16: variant p: rows 32p+d, col 64p+m = W1[m, d]
    W1E = np.zeros((128, 4 * 64), np.float32)
    for p in range(4):
        W1E[32 * p : 32 * p + D, 64 * p : 64 * p + DFF] = W1.T
    c["w1e"] = W1E.astype(ml_dtypes.bfloat16)

    # FFN W2 lhsT bf16: even variant rows 0:48, odd rows 64:112
    W2E = np.zeros((128, 2 * 32), np.float32)
    W2E[0:DFF, 0:D] = W2.T
    W2E[64 : 64 + DFF, 32 : 32 + D] = W2.T
    c["w2e"] = W2E.astype(ml_dtypes.bfloat16)

    c["idtb"] = np.eye(32, dtype=ml_dtypes.bfloat16)

    # banded biases (only used when nonzero)
    GB = np.zeros((128, 2), np.float32)
    for p in range(4):
        GB[32 * p : 32 * p + D, 0] = b1
        GB[32 * p : 32 * p + D, 1] = b2
    c["gb"] = GB
    return c


CONST_SPECS = {
    "wqk1": ((128, 128), BF16),
    "wk2": ((128, 128), BF16),
    "woe": ((128, 32), BF16),
    "cb1": ((128, 128), BF16),
    "cb2": ((128, 128), BF16),
    "selr": ((128, 128), BF16),
    "selg1": ((128, 128), BF16),
    "selg2": ((128, 128), BF16),
    "selb": ((128, 128), BF16),
    "w1e": ((128, 4 * 64), BF16),
    "w2e": ((128, 2 * 32), BF16),
    "idtb": ((32, 32), BF16),
    "gb": ((128, 2), F32),
}


def _pin_act_tables():
    """Pin Exp/Ln to natural_log_exp_and_others so LN's Ln+Exp never thrash."""
    import concourse.bacc as _bacc
    if getattr(_bacc, "_act_tables_pinned", False):
        return
    _orig = _bacc.get_activation_tables

    def _patched(arch):
        tables = dict(_orig(arch))
        keep = "natural_log_exp_and_others"
        for name in list(tables):
            if name != keep and (AF.Exp in tables[name] or AF.Ln in tables[name]):
                tables[name] = set()
        return tables

    _bacc.get_activation_tables = _patched
    _bacc._act_tables_pinned = True


def build_nc(nb: int = NB, use_b: bool = False) -> bass.Bass:
    _pin_act_tables()
    ngroups = nb // 4
    nc = bacc.Bacc()
    x_in = nc.dram_tensor("x", [nb, S, D], F32, kind="ExternalInput")
    out = nc.dram_tensor("out", [nb, S, D], F32, kind="ExternalOutput")
    cin = {
        k: nc.dram_tensor(k, list(sh), dt, kind="ExternalInput")
        for k, (sh, dt) in CONST_SPECS.items()
    }

    with tile.TileContext(nc) as tc:
        import contextlib

        ctx = contextlib.ExitStack()
        with ctx:
            constp = ctx.enter_context(tc.tile_pool(name="consts", bufs=1))
            vdp = ctx.enter_context(tc.tile_pool(name="vdp", bufs=1))
            xnp = ctx.enter_context(tc.tile_pool(name="xn", bufs=2))
            xtp = ctx.enter_context(tc.tile_pool(name="xt", bufs=2))
            xtbp = ctx.enter_context(tc.tile_pool(name="xtb", bufs=2))
            qkp = ctx.enter_context(tc.tile_pool(name="qk", bufs=2))
            ep = ctx.enter_context(tc.tile_pool(name="e", bufs=2))
            uosp = ctx.enter_context(tc.tile_pool(name="uos", bufs=2))
            rcpp = ctx.enter_context(tc.tile_pool(name="rcp", bufs=2))
            otp = ctx.enter_context(tc.tile_pool(name="ot", bufs=2))
            y1p = ctx.enter_context(tc.tile_pool(name="y1", bufs=2))
            lnp = ctx.enter_context(tc.tile_pool(name="ln", bufs=2))
            x1p = ctx.enter_context(tc.tile_pool(name="x1", bufs=2))
            hsp = ctx.enter_context(tc.tile_pool(name="hs", bufs=2))
            fsp = ctx.enter_context(tc.tile_pool(name="fs", bufs=2))
            ytp = ctx.enter_context(tc.tile_pool(name="yt", bufs=2))
            # PSUM: stt 2x[128,1024] + half 2x[128,512] + uo 2x[128,512] = 8 banks
            psA = ctx.enter_context(tc.tile_pool(name="psA", bufs=2, space="PSUM"))
            psB = ctx.enter_context(tc.tile_pool(name="psB", bufs=2, space="PSUM"))
            psC = ctx.enter_context(tc.tile_pool(name="psC", bufs=2, space="PSUM"))

            # prefetch group 0's x before the const DMAs so batch 0's
            # transpose chain overlaps the constant loads
            U0 = xnp.tile([128, 16, 32], F32, name="xu0")
            nc.gpsimd.memset(U0[:, :, D:32], 0.0)
            for pp in range(4):
                nc.sync.dma_start(
                    out=U0[32 * pp : 32 * pp + 32, :, 0:D],
                    in_=x_in[pp].rearrange("(f c) d -> c f d", c=32),
                )
            C = {}
            for k, (sh, dt) in CONST_SPECS.items():
                t = constp.tile(list(sh), dt, name=f"c_{k}")
                nc.sync.dma_start(out=t, in_=cin[k][:, :])
                C[k] = t
            eps_t = constp.tile([128, 1], F32, name="c_eps")
            nc.vector.memset(eps_t, EPS)

            # two persistent VD tiles [128, t4, hp2, i2, m128] fp8 (ping-pong).
            # ones preset at m = 32*(2hp+i) + {0, 7..31} inside each head band
            # (denominator column + finite padding rows for UO).
            vd_tiles = []
            for v in range(2):
                vt = vdp.tile([128, 4, 2, 2, 128], FP8, name=f"vd{v}")
                nc.gpsimd.memset(vt[:, :, :, :, :], 0.0)
                for hp in range(2):
                    for i in range(2):
                        m0 = 32 * (2 * hp + i)
                        nc.gpsimd.memset(vt[:, :, hp, i, m0 : m0 + 1], 1.0)
                        nc.gpsimd.memset(vt[:, :, hp, i, m0 + 7 : m0 + 32], 1.0)
                vd_tiles.append(vt)

            def ln_stages(Y, selg, bslice, OUT, outb=None):
                """LayerNorm over banded d of Y [128,512] fp32r, split into 4
                stages so PE never blocks on the DVE/ACT hops in between."""
                st = {}

                def s1():
                    mps = psB.tile([128, S], F32, name="mps", tag="half")
                    nc.tensor.matmul(mps[:, :], C["cb1"][:, :], Y[:, :],
                                     start=True, stop=True)
                    MU1 = lnp.tile([128, S], BF16, name="mu1", tag="mu")
                    nc.vector.tensor_copy(MU1[:, :], mps[:, :])
                    st["MU1"] = MU1

                def s2():
                    bcmu = psB.tile([128, S], F32, name="bcmu", tag="half")
                    nc.tensor.matmul(bcmu[:, :], C["selr"][:, :], st["MU1"][:, :],
                                     start=True, stop=True)
                    YC = lnp.tile([128, S], F32, name="yc", tag="yc")
                    nc.vector.tensor_add(YC[:, :], Y[:, :], bcmu[:, :])
                    YC2 = lnp.tile([128, S], BF16, name="yc2", tag="yc2")
                    nc.vector.tensor_mul(YC2[:, :], YC[:, :], YC[:, :])
                    st["YC"], st["YC2"] = YC, YC2

                def s3():
                    vps = psB.tile([128, S], F32, name="vps", tag="half")
                    nc.tensor.matmul(vps[:, :], C["cb2"][:, :], st["YC2"][:, :],
                                     start=True, stop=True)
                    LNV = lnp.tile([128, S], F32, name="lnv", tag="lnv")
                    nc.scalar.activation(LNV[:, :], vps[:, :], AF.Ln,
                                         bias=eps_t[:, :])
                    RS1 = lnp.tile([128, S], BF16, name="rs1", tag="rs")
                    nc.scalar.activation(RS1[:, :], LNV[:, :], AF.Exp, scale=-0.5)
                    st["RS1"] = RS1

                def s4():
                    bcrs = psB.tile([128, S], F32, name="bcrs", tag="half")
                    nc.tensor.matmul(bcrs[:, :], selg[:, :], st["RS1"][:, :],
                                     start=True, stop=True)
                    if use_b:
                        TMP = lnp.tile([128, S], F32, name="lntmp", tag="tmp")
                        nc.vector.tensor_mul(TMP[:, :], st["YC"][:, :], bcrs[:, :])
                        nc.vector.tensor_scalar(out=OUT[:, :], in0=TMP[:, :],
                                                scalar1=bslice, scalar2=None,
                                                op0=ALU.add)
                    else:
                        nc.vector.tensor_mul(OUT[:, :], st["YC"][:, :], bcrs[:, :])
                    if outb is not None:
                        nc.gpsimd.tensor_copy(outb[:, :], OUT[:, :])

                return [s1, s2, s3, s4]

            def batch_head(g, p, XT4b, filler):
                """QKV + scores + E-gen for batch 4g+p. Returns state."""
                ps1 = psB.tile([128, S], F32, name="ps1", tag="half")
                nc.tensor.matmul(
                    ps1[:, :], C["wqk1"][32 * p : 32 * p + D, :],
                    XT4b[32 * p : 32 * p + D, :],
                    start=True, stop=True, tile_position=(32 * p, 0),
                )
                ps2 = psB.tile([128, S], F32, name="ps2", tag="half")
                nc.tensor.matmul(
                    ps2[:, :], C["wk2"][32 * p : 32 * p + D, :],
                    XT4b[32 * p : 32 * p + D, :],
                    start=True, stop=True, tile_position=(32 * p, 0),
                )
                QK = qkp.tile([128, 2 * S], BF16, name="qk")
                nc.vector.tensor_copy(QK[:, 0:S], ps1[:, :])
                nc.scalar.activation(QK[:, S : 2 * S], ps2[:, :], AF.Copy)

                # V^T via PE transposes of rows 0:32 (V lives at rows 8:32)
                psv = psB.tile([128, 4 * 32], BF16, name="psv", tag="half")
                for t in range(4):
                    nc.tensor.transpose(
                        psv[:, 32 * t : 32 * (t + 1)],
                        QK[0:32, 128 * t : 128 * (t + 1)],
                        C["idtb"][:, :],
                    )
                VD = vd_tiles[(4 * g + p) % 2]
                # VD[k, t, hp, i, 32*(2hp+i)+1 : +7] = psv[k, 32t+8+6(2hp+i) : +6]
                vd_dst = bass.AP(
                    tensor=VD.tensor, offset=VD.offset + 1,
                    ap=[list(VD.ap[0]), [512, 4], [320, 2], [160, 2], [1, 6]],
                )
                vd_src = bass.AP(
                    tensor=psv.tensor, offset=psv.offset + 8,
                    ap=[list(psv.ap[0]), [32, 4], [12, 2], [6, 2], [1, 6]],
                )
                nc.vector.tensor_copy(vd_dst, vd_src)

                # scores (S^T, raw) + E-gen (fp8e4m3 bits via affine).
                # After each score pair, pop one filler closure (AV of the
                # previous batch / tail matmuls / group stages) so the PE
                # queue always has ready work between stt slot waits.
                E = ep.tile([128, 4, 2, 2, S], FP8, name="e")
                for t in range(4):
                    for hp in range(2):
                        stt = psA.tile([128, 2 * S], F32, name="stt", tag="big")
                        for i in range(2):
                            h = 2 * hp + i
                            nc.tensor.matmul(
                                stt[:, 512 * i : 512 * (i + 1)],
                                QK[32 * h : 32 * h + HD,
                                   S + 128 * t : S + 128 * (t + 1)],
                                QK[32 * h : 32 * h + HD, 0:S],
                                start=True, stop=True,
                                tile_position=(32 * h, 0),
                                skip_group_check=True,
                            )
                        eng = EGEN_ENGINES[2 * t + hp]
                        e_out = E[:, t, hp, :, :].bitcast(U8)
                        s_in = stt.rearrange("p (i q) -> p i q", i=2)
                        if eng == "act":
                            nc.scalar.activation(e_out, s_in, AF.Copy,
                                                 bias=EB, scale=EA)
                        else:
                            nc.vector.tensor_scalar(
                                out=e_out, in0=s_in, scalar1=EA, scalar2=EB,
                                op0=ALU.mult, op1=ALU.add,
                            )
                        if filler:
                            filler.pop(0)()

                return {"E": E, "VD": VD, "p": p}

            def av_closures(state):
                """fp8 DoubleRow AV as 8 single-matmul closures (interleavable)."""
                E, VD = state["E"], state["VD"]
                UO = psC.tile([128, S], F32, name="uo", tag="uo")
                state["UO"] = UO

                def mk(t, hp, idx):
                    def go():
                        nc.tensor.matmul(
                            UO[:, :], VD[:, t, hp, :, :], E[:, t, hp, :, :],
                            start=(idx == 0), stop=(idx == 7),
                            perf_mode=DR, tile_position=(0, 0),
                            skip_group_check=True,
                        )
                    return go

                return [mk(t, hp, 2 * t + hp) for t in range(4) for hp in range(2)]

            def tail_closures(state, XT4, Y1):
                """normalize + Wo + residual for a batch, as two closures."""
                p = state["p"]
                hold = {}

                def t1():
                    UOs = uosp.tile([128, S], BF16, name="uos")
                    nc.scalar.activation(UOs[:, :], state["UO"][:, :], AF.Copy)
                    denps = psB.tile([128, S], F32, name="denps", tag="half")
                    nc.tensor.matmul(denps[:, :], C["selb"][:, :], UOs[:, :],
                                     start=True, stop=True)
                    hold["UOs"], hold["denps"] = UOs, denps

                def t2a():
                    RCP = rcpp.tile([128, S], F32, name="rcpt")
                    nc.vector.reciprocal_approx_fast(RCP[:, :], hold["denps"][:, :])
                    OTn = otp.tile([128, S], BF16, name="otn")
                    nc.vector.tensor_mul(OTn[:, :], hold["UOs"][:, :], RCP[:, :])
                    hold["OTn"] = OTn

                def t2b():
                    wops = psB.tile([32, S], F32, name="wops", tag="half")
                    nc.tensor.matmul(wops[:, :], C["woe"][:, :], hold["OTn"][:, :],
                                     start=True, stop=True)
                    nc.vector.tensor_add(
                        Y1[32 * p : 32 * p + 32, :], wops[:, :],
                        XT4[32 * p : 32 * p + 32, :],
                    )

                return [t1, t2a, t2b]

            def group_stages(g, Y1):
                """LN1 + FFN + LN2 + output store for group g as a stage list."""
                X1 = x1p.tile([128, S], BF16, name="x1")
                stages = ln_stages(Y1, C["selg1"], C["gb"][:, 0:1], X1)
                st = {}

                def ffn_a():
                    F4 = psC.tile([128, S], F32, name="f4", tag="uo")
                    hps = psB.tile([128, S], F32, name="hps", tag="half")
                    for j in range(2):
                        nc.tensor.matmul(
                            hps[64 * j : 64 * j + 64, :],
                            C["w1e"][:, 64 * j : 64 * (j + 1)],
                            X1[:, :],
                            start=True, stop=True, tile_position=(0, 64 * j),
                            skip_group_check=True,
                        )
                    HS = hsp.tile([128, S], BF16, name="hs")
                    nc.scalar.activation(HS[:, :], hps[:, :], AF.Relu)
                    st["F4"], st["HS"] = F4, HS

                def ffn_b():
                    F4 = st["F4"]
                    for j in range(2):
                        nc.tensor.matmul(
                            F4[32 * j : 32 * j + 32, :],
                            C["w2e"][:, 32 * j : 32 * (j + 1)],
                            st["HS"][:, :],
                            start=True, stop=True, tile_position=(0, 32 * j),
                            skip_group_check=True,
                        )
                    hps = psB.tile([128, S], F32, name="hps2", tag="half")
                    for j in range(2):
                        nc.tensor.matmul(
                            hps[64 * j : 64 * j + 64, :],
                            C["w1e"][:, 64 * (2 + j) : 64 * (3 + j)],
                            X1[:, :],
                            start=True, stop=True, tile_position=(0, 64 * j),
                            skip_group_check=True,
                        )
                    HS2 = hsp.tile([128, S], BF16, name="hs2", tag="hs2")
                    nc.scalar.activation(HS2[:, :], hps[:, :], AF.Relu)
                    st["HS2"] = HS2

                def ffn_c():
                    F4 = st["F4"]
                    for j in range(2):
                        nc.tensor.matmul(
                            F4[32 * (2 + j) : 32 * (3 + j), :],
                            C["w2e"][:, 32 * j : 32 * (j + 1)],
                            st["HS2"][:, :],
                            start=True, stop=True, tile_position=(0, 32 * (2 + j)),
                            skip_group_check=True,
                        )
                    FS0 = fsp.tile([128, S], BF16, name="fs0", tag="fs0")
                    nc.scalar.activation(FS0[:, :], F4[:, :], AF.Relu)
                    FS = fsp.tile([128, S], BF16, name="fst", tag="fst")
                    nc.gpsimd.tensor_add(FS[:, :], FS0[:, :], X1[:, :])
                    st["FS"] = FS

                stages += [ffn_a, ffn_b, ffn_c]

                Y2N = ytp.tile([128, S], F32, name="y2n", tag="y2n")
                ln2 = [None]

                def ln2_s1():
                    ln2[0] = ln_stages(st["FS"], C["selg2"], C["gb"][:, 1:2], Y2N)
                    ln2[0][0]()

                def emit_out():
                    Y2T = ytp.tile([128, S], F32, name="y2t", tag="y2t")
                    nc.vector.transpose(Y2T[:, :], Y2N[:, :])
                    for p in range(4):
                        b = 4 * g + p
                        nc.sync.dma_start(
                            out=out[b].rearrange("(f r) d -> r f d", r=32),
                            in_=Y2T[32 * p : 32 * p + 32, :].rearrange(
                                "r (f c) -> r f c", c=32
                            )[:, :, 0:D],
                        )

                stages += [ln2_s1,
                           lambda: ln2[0][1](),
                           lambda: ln2[0][2](),
                           lambda: ln2[0][3](),
                           emit_out]
                return stages

            # slot machine: headA(i) | AV(i-1) | tail(i-2) | <=3 group stages
            states = {}
            gctx = {}
            pending = []
            nslots = 4 * ngroups

            def emit_slot(i):
                g, p = divmod(i, 4)
                if p == 0:
                    if g == 0:
                        U = U0
                    else:
                        U = xnp.tile([128, 16, 32], F32, name="xu")
                        nc.gpsimd.memset(U[:, :, D:32], 0.0)
                        for pp in range(4):
                            nc.sync.dma_start(
                                out=U[32 * pp : 32 * pp + 32, :, 0:D],
                                in_=x_in[4 * g + pp].rearrange("(f c) d -> c f d", c=32),
                            )
                    XT4 = xtp.tile([128, S], F32, name="xt4")
                    nc.vector.transpose(XT4[:, :], U.rearrange("p a b -> p (a b)"))
                    XT4b = xtbp.tile([128, S], BF16, name="xt4b")
                    nc.vector.tensor_copy(XT4b[:, :], XT4[:, :])
                    Y1 = y1p.tile([128, S], BF16, name="y1")
                    gctx[g] = (XT4, XT4b, Y1)
                XT4, XT4b, Y1 = gctx[g]
                base = []
                post = []
                gpush = None
                if i - 1 in states:
                    base += av_closures(states[i - 1])
                if i - 2 in states:
                    s = states.pop(i - 2)
                    g2 = (i - 2) // 4
                    tcs = tail_closures(s, gctx[g2][0], gctx[g2][2])
                    base += tcs[:2]
                    post.append(tcs[2])
                    if (i - 2) % 4 == 3:
                        gpush = g2   # push AFTER t2b writes Y1's last band
                gs = []
                for _ in range(3):
                    if pending:
                        gs.append(pending.pop(0))
                # weave group stages ~4 filler positions apart so their
                # dependency chains clear before the next stage's PE matmul
                filler = (base[0:3] + gs[0:1] + base[3:6] + gs[1:2]
                          + base[6:9] + gs[2:3] + base[9:])
                states[i] = batch_head(g, p, XT4b, filler)
                while filler:
                    filler.pop(0)()
                for f in post:
                    f()
                if gpush is not None:
                    pending.extend(group_stages(gpush, gctx[gpush][2]))

            for i in range(nslots):
                emit_slot(i)
            # drain
            for f in av_closures(states[nslots - 1]):
                f()
            for i in (nslots - 2, nslots - 1):
                s = states.pop(i)
                g2 = i // 4
                for f in tail_closures(s, gctx[g2][0], gctx[g2][2]):
                    f()
                if i % 4 == 3:
                    pending.extend(group_stages(g2, gctx[g2][2]))
            while pending:
                pending.pop(0)()

    nc.compile()
    return nc


def build_nc(nb: int = NB, use_b: bool = False) -> bass.Bass:
    _pin_act_tables()
    ngroups = nb // 4
    nc = bacc.Bacc()
    x_in = nc.dram_tensor("x", [nb, S, D], F32, kind="ExternalInput")
    out = nc.dram_tensor("out", [nb, S, D], F32, kind="ExternalOutput")
    cin = {
        k: nc.dram_tensor(k, list(sh), dt, kind="ExternalInput")
        for k, (sh, dt) in CONST_SPECS.items()
    }

    with tile.TileContext(nc) as tc:
        import contextlib

        ctx = contextlib.ExitStack()
        with ctx:
            constp = ctx.enter_context(tc.tile_pool(name="consts", bufs=1))
            vdp = ctx.enter_context(tc.tile_pool(name="vdp", bufs=1))
            xnp = ctx.enter_context(tc.tile_pool(name="xn", bufs=2))
            xtp = ctx.enter_context(tc.tile_pool(name="xt", bufs=2))
            xtbp = ctx.enter_context(tc.tile_pool(name="xtb", bufs=2))
            qkp = ctx.enter_context(tc.tile_pool(name="qk", bufs=2))
            ep = ctx.enter_context(tc.tile_pool(name="e", bufs=2))
            uosp = ctx.enter_context(tc.tile_pool(name="uos", bufs=2))
            rcpp = ctx.enter_context(tc.tile_pool(name="rcp", bufs=2))
            otp = ctx.enter_context(tc.tile_pool(name="ot", bufs=2))
            y1p = ctx.enter_context(tc.tile_pool(name="y1", bufs=2))
            lnp = ctx.enter_context(tc.tile_pool(name="ln", bufs=2))
            x1p = ctx.enter_context(tc.tile_pool(name="x1", bufs=2))
            hsp = ctx.enter_context(tc.tile_pool(name="hs", bufs=2))
            fsp = ctx.enter_context(tc.tile_pool(name="fs", bufs=2))
            ytp = ctx.enter_context(tc.tile_pool(name="yt", bufs=2))
            # PSUM: stt 2x[128,1024] + half 2x[128,512] + uo 2x[128,512] = 8 banks
            psA = ctx.enter_context(tc.tile_pool(name="psA", bufs=2, space="PSUM"))
            psB = ctx.enter_context(tc.tile_pool(name="psB", bufs=2, space="PSUM"))
            psC = ctx.enter_context(tc.tile_pool(name="psC", bufs=2, space="PSUM"))

            # prefetch group 0's x before the const DMAs so batch 0's
            # transpose chain overlaps the constant loads
            U0 = xnp.tile([128, 16, 32], F32, name="xu0")
            nc.gpsimd.memset(U0[:, :, D:32], 0.0)
            for pp in range(4):
                nc.sync.dma_start(
                    out=U0[32 * pp : 32 * pp + 32, :, 0:D],
                    in_=x_in[pp].rearrange("(f c) d -> c f d", c=32),
                )
            C = {}
            for k, (sh, dt) in CONST_SPECS.items():
                t = constp.tile(list(sh), dt, name=f"c_{k}")
                nc.sync.dma_start(out=t, in_=cin[k][:, :])
                C[k] = t
            eps_t = constp.tile([128, 1], F32, name="c_eps")
            nc.vector.memset(eps_t, EPS)

            # two persistent VD tiles [128, t4, hp2, i2, m128] fp8 (ping-pong).
            # ones preset at m = 32*(2hp+i) + {0, 7..31} inside each head band
            # (denominator column + finite padding rows for UO).
            vd_tiles = []
            for v in range(2):
                vt = vdp.tile([128, 4, 2, 2, 128], FP8, name=f"vd{v}")
                nc.gpsimd.memset(vt[:, :, :, :, :], 0.0)
                for hp in range(2):
                    for i in range(2):
                        m0 = 32 * (2 * hp + i)
                        nc.gpsimd.memset(vt[:, :, hp, i, m0 : m0 + 1], 1.0)
                        nc.gpsimd.memset(vt[:, :, hp, i, m0 + 7 : m0 + 32], 1.0)
                vd_tiles.append(vt)

            def ln_stages(Y, selg, bslice, OUT, outb=None):
                """LayerNorm over banded d of Y [128,512] fp32r, split into 4
                stages so PE never blocks on the DVE/ACT hops in between."""
                st = {}

                def s1():
                    mps = psB.tile([128, S], F32, name="mps", tag="half")
                    nc.tensor.matmul(mps[:, :], C["cb1"][:, :], Y[:, :],
                                     start=True, stop=True)
                    MU1 = lnp.tile([128, S], BF16, name="mu1", tag="mu")
                    nc.vector.tensor_copy(MU1[:, :], mps[:, :])
                    st["MU1"] = MU1

                def s2():
                    bcmu = psB.tile([128, S], F32, name="bcmu", tag="half")
                    nc.tensor.matmul(bcmu[:, :], C["selr"][:, :], st["MU1"][:, :],
                                     start=True, stop=True)
                    YC = lnp.tile([128, S], F32, name="yc", tag="yc")
                    nc.vector.tensor_add(YC[:, :], Y[:, :], bcmu[:, :])
                    YC2 = lnp.tile([128, S], BF16, name="yc2", tag="yc2")
                    nc.vector.tensor_mul(YC2[:, :], YC[:, :], YC[:, :])
                    st["YC"], st["YC2"] = YC, YC2

                def s3():
                    vps = psB.tile([128, S], F32, name="vps", tag="half")
                    nc.tensor.matmul(vps[:, :], C["cb2"][:, :], st["YC2"][:, :],
                                     start=True, stop=True)
                    LNV = lnp.tile([128, S], F32, name="lnv", tag="lnv")
                    nc.scalar.activation(LNV[:, :], vps[:, :], AF.Ln,
                                         bias=eps_t[:, :])
                    RS1 = lnp.tile([128, S], BF16, name="rs1", tag="rs")
                    nc.scalar.activation(RS1[:, :], LNV[:, :], AF.Exp, scale=-0.5)
                    st["RS1"] = RS1

                def s4():
                    bcrs = psB.tile([128, S], F32, name="bcrs", tag="half")
                    nc.tensor.matmul(bcrs[:, :], selg[:, :], st["RS1"][:, :],
                                     start=True, stop=True)
                    if use_b:
                        TMP = lnp.tile([128, S], F32, name="lntmp", tag="tmp")
                        nc.vector.tensor_mul(TMP[:, :], st["YC"][:, :], bcrs[:, :])
                        nc.vector.tensor_scalar(out=OUT[:, :], in0=TMP[:, :],
                                                scalar1=bslice, scalar2=None,
                                                op0=ALU.add)
                    else:
                        nc.vector.tensor_mul(OUT[:, :], st["YC"][:, :], bcrs[:, :])
                    if outb is not None:
                        nc.gpsimd.tensor_copy(outb[:, :], OUT[:, :])

                return [s1, s2, s3, s4]

            def batch_head(g, p, XT4b, filler):
                """QKV + scores + E-gen for batch 4g+p. Returns state."""
                ps1 = psB.tile([128, S], F32, name="ps1", tag="half")
                nc.tensor.matmul(
                    ps1[:, :], C["wqk1"][32 * p : 32 * p + D, :],
                    XT4b[32 * p : 32 * p + D, :],
                    start=True, stop=True, tile_position=(32 * p, 0),
                )
                ps2 = psB.tile([128, S], F32, name="ps2", tag="half")
                nc.tensor.matmul(
                    ps2[:, :], C["wk2"][32 * p : 32 * p + D, :],
                    XT4b[32 * p : 32 * p + D, :],
                    start=True, stop=True, tile_position=(32 * p, 0),
                )
                QK = qkp.tile([128, 2 * S], BF16, name="qk")
                nc.vector.tensor_copy(QK[:, 0:S], ps1[:, :])
                nc.scalar.activation(QK[:, S : 2 * S], ps2[:, :], AF.Copy)

                # V^T via PE transposes of rows 0:32 (V lives at rows 8:32)
                psv = psB.tile([128, 4 * 32], BF16, name="psv", tag="half")
                for t in range(4):
                    nc.tensor.transpose(
                        psv[:, 32 * t : 32 * (t + 1)],
                        QK[0:32, 128 * t : 128 * (t + 1)],
                        C["idtb"][:, :],
                    )
                VD = vd_tiles[(4 * g + p) % 2]
                # VD[k, t, hp, i, 32*(2hp+i)+1 : +7] = psv[k, 32t+8+6(2hp+i) : +6]
                vd_dst = bass.AP(
                    tensor=VD.tensor, offset=VD.offset + 1,
                    ap=[list(VD.ap[0]), [512, 4], [320, 2], [160, 2], [1, 6]],
                )
                vd_src = bass.AP(
                    tensor=psv.tensor, offset=psv.offset + 8,
                    ap=[list(psv.ap[0]), [32, 4], [12, 2], [6, 2], [1, 6]],
                )
                nc.vector.tensor_copy(vd_dst, vd_src)

                # scores (S^T, raw) + E-gen (fp8e4m3 bits via affine).
                # After each score pair, pop one filler closure (AV of the
                # previous batch / tail matmuls / group stages) so the PE
                # queue always has ready work between stt slot waits.
                E = ep.tile([128, 4, 2, 2, S], FP8, name="e")
                for t in range(4):
                    for hp in range(2):
                        stt = psA.tile([128, 2 * S], F32, name="stt", tag="big")
                        for i in range(2):
                            h = 2 * hp + i
                            nc.tensor.matmul(
                                stt[:, 512 * i : 512 * (i + 1)],
                                QK[32 * h : 32 * h + HD,
                                   S + 128 * t : S + 128 * (t + 1)],
                                QK[32 * h : 32 * h + HD, 0:S],
                                start=True, stop=True,
                                tile_position=(32 * h, 0),
                                skip_group_check=True,
                            )
                        eng = EGEN_ENGINES[2 * t + hp]
                        e_out = E[:, t, hp, :, :].bitcast(U8)
                        s_in = stt.rearrange("p (i q) -> p i q", i=2)
                        if eng == "act":
                            nc.scalar.activation(e_out, s_in, AF.Copy,
                                                 bias=EB, scale=EA)
                        else:
                            nc.vector.tensor_scalar(
                                out=e_out, in0=s_in, scalar1=EA, scalar2=EB,
                                op0=ALU.mult, op1=ALU.add,
                            )
                        if filler:
                            filler.pop(0)()

                return {"E": E, "VD": VD, "p": p}

            def av_closures(state):
                """fp8 DoubleRow AV as 8 single-matmul closures (interleavable)."""
                E, VD = state["E"], state["VD"]
                UO = psC.tile([128, S], F32, name="uo", tag="uo")
                state["UO"] = UO

                def mk(t, hp, idx):
                    def go():
                        nc.tensor.matmul(
                            UO[:, :], VD[:, t, hp, :, :], E[:, t, hp, :, :],
                            start=(idx == 0), stop=(idx == 7),
                            perf_mode=DR, tile_position=(0, 0),
                            skip_group_check=True,
                        )
                    return go

                return [mk(t, hp, 2 * t + hp) for t in range(4) for hp in range(2)]

            def tail_closures(state, XT4, Y1):
                """normalize + Wo + residual for a batch, as two closures."""
                p = state["p"]
                hold = {}

                def t1():
                    UOs = uosp.tile([128, S], BF16, name="uos")
                    nc.scalar.activation(UOs[:, :], state["UO"][:, :], AF.Copy)
                    denps = psB.tile([128, S], F32, name="denps", tag="half")
                    nc.tensor.matmul(denps[:, :], C["selb"][:, :], UOs[:, :],
                                     start=True, stop=True)
                    hold["UOs"], hold["denps"] = UOs, denps

                def t2a():
                    RCP = rcpp.tile([128, S], F32, name="rcpt")
                    nc.vector.reciprocal_approx_fast(RCP[:, :], hold["denps"][:, :])
                    OTn = otp.tile([128, S], BF16, name="otn")
                    nc.vector.tensor_mul(OTn[:, :], hold["UOs"][:, :], RCP[:, :])
                    hold["OTn"] = OTn

                def t2b():
                    wops = psB.tile([32, S], F32, name="wops", tag="half")
                    nc.tensor.matmul(wops[:, :], C["woe"][:, :], hold["OTn"][:, :],
                                     start=True, stop=True)
                    nc.vector.tensor_add(
                        Y1[32 * p : 32 * p + 32, :], wops[:, :],
                        XT4[32 * p : 32 * p + 32, :],
                    )

                return [t1, t2a, t2b]

            def group_stages(g, Y1):
                """LN1 + FFN + LN2 + output store for group g as a stage list."""
                X1 = x1p.tile([128, S], BF16, name="x1")
                stages = ln_stages(Y1, C["selg1"], C["gb"][:, 0:1], X1)
                st = {}

                def ffn_a():
                    F4 = psC.tile([128, S], F32, name="f4", tag="uo")
                    hps = psB.tile([128, S], F32, name="hps", tag="half")
                    for j in range(2):
                        nc.tensor.matmul(
                            hps[64 * j : 64 * j + 64, :],
                            C["w1e"][:, 64 * j : 64 * (j + 1)],
                            X1[:, :],
                            start=True, stop=True, tile_position=(0, 64 * j),
                            skip_group_check=True,
                        )
                    HS = hsp.tile([128, S], BF16, name="hs")
                    nc.scalar.activation(HS[:, :], hps[:, :], AF.Relu)
                    st["F4"], st["HS"] = F4, HS

                def ffn_b():
                    F4 = st["F4"]
                    for j in range(2):
                        nc.tensor.matmul(
                            F4[32 * j : 32 * j + 32, :],
                            C["w2e"][:, 32 * j : 32 * (j + 1)],
                            st["HS"][:, :],
                            start=True, stop=True, tile_position=(0, 32 * j),
                            skip_group_check=True,
                        )
                    hps = psB.tile([128, S], F32, name="hps2", tag="half")
                    for j in range(2):
                        nc.tensor.matmul(
                            hps[64 * j : 64 * j + 64, :],
                            C["w1e"][:, 64 * (2 + j) : 64 * (3 + j)],
                            X1[:, :],
                            start=True, stop=True, tile_position=(0, 64 * j),
                            skip_group_check=True,
                        )
                    HS2 = hsp.tile([128, S], BF16, name="hs2", tag="hs2")
                    nc.scalar.activation(HS2[:, :], hps[:, :], AF.Relu)
                    st["HS2"] = HS2

                def ffn_c():
                    F4 = st["F4"]
                    for j in range(2):
                        nc.tensor.matmul(
                            F4[32 * (2 + j) : 32 * (3 + j), :],
                            C["w2e"][:, 32 * j : 32 * (j + 1)],
                            st["HS2"][:, :],
                            start=True, stop=True, tile_position=(0, 32 * (2 + j)),
                            skip_group_check=True,
                        )
                    FS0 = fsp.tile([128, S], BF16, name="fs0", tag="fs0")
                    nc.scalar.activation(FS0[:, :], F4[:, :], AF.Relu)
                    FS = fsp.tile([128, S], BF16, name="fst", tag="fst")
                    nc.gpsimd.tensor_add(FS[:, :], FS0[:, :], X1[:, :])
                    st["FS"] = FS

                stages += [ffn_a, ffn_b, ffn_c]

                Y2N = ytp.tile([128, S], F32, name="y2n", tag="y2n")
                ln2 = [None]

                def ln2_s1():
                    ln2[0] = ln_stages(st["FS"], C["selg2"], C["gb"][:, 1:2], Y2N)
                    ln2[0][0]()

                def emit_out():
                    Y2T = ytp.tile([128, S], F32, name="y2t", tag="y2t")
                    nc.vector.transpose(Y2T[:, :], Y2N[:, :])
                    for p in range(4):
                        b = 4 * g + p
                        nc.sync.dma_start(
                            out=out[b].rearrange("(f r) d -> r f d", r=32),
                            in_=Y2T[32 * p : 32 * p + 32, :].rearrange(
                                "r (f c) -> r f c", c=32
                            )[:, :, 0:D],
                        )

                stages += [ln2_s1,
                           lambda: ln2[0][1](),
                           lambda: ln2[0][2](),
                           lambda: ln2[0][3](),
                           emit_out]
                return stages

            # slot machine: headA(i) | AV(i-1) | tail(i-2) | <=3 group stages
            states = {}
            gctx = {}
            pending = []
            nslots = 4 * ngroups

            def emit_slot(i):
                g, p = divmod(i, 4)
                if p == 0:
                    if g == 0:
                        U = U0
                    else:
                        U = xnp.tile([128, 16, 32], F32, name="xu")
                        nc.gpsimd.memset(U[:, :, D:32], 0.0)
                        for pp in range(4):
                            nc.sync.dma_start(
                                out=U[32 * pp : 32 * pp + 32, :, 0:D],
                                in_=x_in[4 * g + pp].rearrange("(f c) d -> c f d", c=32),
                            )
                    XT4 = xtp.tile([128, S], F32, name="xt4")
                    nc.vector.transpose(XT4[:, :], U.rearrange("p a b -> p (a b)"))
                    XT4b = xtbp.tile([128, S], BF16, name="xt4b")
                    nc.vector.tensor_copy(XT4b[:, :], XT4[:, :])
                    Y1 = y1p.tile([128, S], BF16, name="y1")
                    gctx[g] = (XT4, XT4b, Y1)
                XT4, XT4b, Y1 = gctx[g]
                base = []
                post = []
                gpush = None
                if i - 1 in states:
                    base += av_closures(states[i - 1])
                if i - 2 in states:
                    s = states.pop(i - 2)
                    g2 = (i - 2) // 4
                    tcs = tail_closures(s, gctx[g2][0], gctx[g2][2])
                    base += tcs[:2]
                    post.append(tcs[2])
                    if (i - 2) % 4 == 3:
                        gpush = g2   # push AFTER t2b writes Y1's last band
                gs = []
                for _ in range(3):
                    if pending:
                        gs.append(pending.pop(0))
                # weave group stages ~4 filler positions apart so their
                # dependency chains clear before the next stage's PE matmul
                filler = (base[0:3] + gs[0:1] + base[3:6] + gs[1:2]
                          + base[6:9] + gs[2:3] + base[9:])
                states[i] = batch_head(g, p, XT4b, filler)
                while filler:
                    filler.pop(0)()
                for f in post:
                    f()
                if gpush is not None:
                    pending.extend(group_stages(gpush, gctx[gpush][2]))

            for i in range(nslots):
                emit_slot(i)
            # drain
            for f in av_closures(states[nslots - 1]):
                f()
            for i in (nslots - 2, nslots - 1):
                s = states.pop(i)
                g2 = i // 4
                for f in tail_closures(s, gctx[g2][0], gctx[g2][2]):
                    f()
                if i % 4 == 3:
                    pending.extend(group_stages(g2, gctx[g2][2]))
            while pending:
                pending.pop(0)()

    nc.compile()
    return nc


def _enable_ldw_opt():
    """Flip walrus --enable-ldw-opt to true (hides LDWEIGHTS behind matmuls)."""
    import concourse.bass_utils as _bu
    if getattr(_bu, "_ldw_opt_patched", False):
        return
    _orig = _bu.run_command

    def _patched(cmd, *a, **kw):
        if isinstance(cmd, list):
            cmd = ["--enable-ldw-opt=true" if c == "--enable-ldw-opt=false" else c
                   for c in cmd]
        return _orig(cmd, *a, **kw)

    _bu.run_command = _patched
    _bu._ldw_opt_patched = True


_NC_CACHE: dict[tuple, bass.Bass] = {}


def _get_nc(nb: int, use_b: bool = False) -> bass.Bass:
    key = (nb, use_b)
    if key not in _NC_CACHE:
        _NC_CACHE[key] = build_nc(nb, use_b)
    return _NC_CACHE[key]


def kernel(x, Wq, Wk, Wv, Wo, W1, W2, g1, b1, g2, b2):
    x = np.asarray(x, np.float32)
    args = [np.asarray(a, np.float32) for a in (Wq, Wk, Wv, Wo, W1, W2, g1, b1, g2, b2)]
    consts = _host_consts(*args)
    use_b = bool(np.any(args[7]) or np.any(args[9]))
    nc = _get_nc(NB, use_b)
    in_maps = []
    for c in range(NCORES):
        m = {"x": np.ascontiguousarray(x[c * NB : (c + 1) * NB])}
        m.update(consts)
        in_maps.append(m)
    res = run_bass_kernel_spmd(nc, in_maps, list(range(NCORES)))
    return np.concatenate([r["out"] for r in res.results], axis=0)


# revision 23
# speedup vs baseline: 1.2505x; 1.0148x over previous
"""Trainium2 Bass kernel for nn_Encoder_block (B=128,S=512,D=24,H=4,HD=6,DFF=48).

Data parallel over batch: 16 batches/core x 8 cores. Per core, T-layout
([d, token], d on partitions) with 4 batches banded per 128 partitions.

v2 speedups over the 478us baseline:
  - bf16 QKV/Wo matmuls and fp32r LN/FFN matmuls (1 cyc/row instead of 4).
  - softmax exp is a single Schraudolph-style affine per score pair: fp8e4m3
    BITS = round(s_raw * 8*log2e/sqrt(6) + 55.66) computed by one
    tensor_scalar/activation into a uint8-bitcast of the E tile. Replaces
    exact ACT exp + separate fp8 quantize.
  - AV uses fp8 DoubleRow matmuls whose two planes carry two HEADS via
    block-diagonal V weights: 8 matmuls x 512 cols x 0.5 cyc covers the whole
    attention-value product, landing directly in banded T-layout with the
    softmax denominators riding along as ones-columns.
  - softmax normalize: ACT copy UO->bf16, PE selector-broadcast of the
    denominator row, DVE reciprocal + multiply (no DMA broadcasts).
  - LayerNorm: selector matmuls for mean/var, PE broadcast of -mu and
    g*rstd (g folded into the selector weights), everything fp32r.
  - per-batch "tail" (normalize+Wo) emitted one batch late so PE stays busy.
"""

import os
import sys

import numpy as np

for _p in ("/opt/trn_rl_repo", "/opt/trn_rl_repo/concourse"):
    if os.path.isdir(_p) and _p not in sys.path:
        sys.path.insert(0, _p)

import concourse.bass as bass
import concourse.bacc as bacc
import concourse.mybir as mybir
import concourse.tile as tile
from concourse.bass_utils import run_bass_kernel_spmd

F32 = mybir.dt.float32
F32R = mybir.dt.float32r
BF16 = mybir.dt.bfloat16
FP8 = mybir.dt.float8e4
U8 = mybir.dt.uint8
AF = mybir.ActivationFunctionType
ALU = mybir.AluOpType
DR = mybir.MatmulPerfMode.DoubleRow

B, S, D = 128, 512, 24
H, HD, DFF = 4, 6, 48
EPS = 1e-5
NCORES = 8
NB = B // NCORES          # batches per core = 16
SCALE = 1.0 / np.sqrt(HD)
EA = float(8.0 * np.log2(np.e) * SCALE)   # fp8e4m3 bits slope
EB = 55.66                                # fp8e4m3 bits offset (calibrated)

# E-gen engine per (t, hp) slot: balance ACT vs DVE load
EGEN_ENGINES = ["act", "dve", "act", "dve", "act", "dve", "act", "act"]


def _host_consts(Wq, Wk, Wv, Wo, W1, W2, g1, b1, g2, b2):
    import ml_dtypes
    c = {}
    # QKV lhsT (bf16): per band p: col 32h+j = Wq[6h+j,:], cols 8:32 = Wv rows
    wqk1 = np.zeros((D, 128), np.float32)
    wk2 = np.zeros((D, 128), np.float32)
    for h in range(H):
        for j in range(HD):
            wqk1[:, 32 * h + j] = Wq[6 * h + j, :]
            wk2[:, 32 * h + j] = Wk[6 * h + j, :]
    for dv in range(D):
        wqk1[:, 8 + dv] = Wv[dv, :]
    WQK1 = np.zeros((128, 128), np.float32)
    WK2 = np.zeros((128, 128), np.float32)
    for p in range(4):
        WQK1[32 * p : 32 * p + D, :] = wqk1
        WK2[32 * p : 32 * p + D, :] = wk2
    c["wqk1"] = WQK1.astype(ml_dtypes.bfloat16)
    c["wk2"] = WK2.astype(ml_dtypes.bfloat16)

    # Wo lhsT bf16: rows 32h+1+j = Wo[:, 6h+j] (row 32h is the denominator)
    WOE = np.zeros((128, 32), np.float32)
    for h in range(H):
        for j in range(HD):
            WOE[32 * h + 1 + j, 0:D] = Wo[:, 6 * h + j]
    c["woe"] = WOE.astype(ml_dtypes.bfloat16)

    # LN selectors (fp32r): cb1 col 32p = -1/24 over band p; cb2 = +1/24
    CB1 = np.zeros((128, 128), np.float32)
    CB2 = np.zeros((128, 128), np.float32)
    for p in range(4):
        CB1[32 * p : 32 * p + D, 32 * p] = -1.0 / D
        CB2[32 * p : 32 * p + D, 32 * p] = 1.0 / D
    c["cb1"] = CB1.astype(ml_dtypes.bfloat16)
    c["cb2"] = CB2.astype(ml_dtypes.bfloat16)

    # broadcast selectors: col m -> 1 at row 32*(m//32); selg folds g
    SELR = np.zeros((128, 128), np.float32)
    SELG1 = np.zeros((128, 128), np.float32)
    SELG2 = np.zeros((128, 128), np.float32)
    for m in range(128):
        SELR[32 * (m // 32), m] = 1.0
        if m % 32 < D:
            SELG1[32 * (m // 32), m] = g1[m % 32]
            SELG2[32 * (m // 32), m] = g2[m % 32]
    c["selr"] = SELR.astype(ml_dtypes.bfloat16)
    c["selg1"] = SELG1.astype(ml_dtypes.bfloat16)
    c["selg2"] = SELG2.astype(ml_dtypes.bfloat16)
    c["selb"] = SELR.astype(ml_dtypes.bfloat16)

    # FFN W1 lhsT bf16: variant p: rows 32p+d, col 64p+m = W1[m, d]
    W1E = np.zeros((128, 4 * 64), np.float32)
    for p in range(4):
        W1E[32 * p : 32 * p + D, 64 * p : 64 * p + DFF] = W1.T
    c["w1e"] = W1E.astype(ml_dtypes.bfloat16)

    # FFN W2 lhsT bf16: even variant rows 0:48, odd rows 64:112
    W2E = np.zeros((128, 2 * 32), np.float32)
    W2E[0:DFF, 0:D] = W2.T
    W2E[64 : 64 + DFF, 32 : 32 + D] = W2.T
    c["w2e"] = W2E.astype(ml_dtypes.bfloat16)

    c["idtb"] = np.eye(32, dtype=ml_dtypes.bfloat16)

    # banded biases (only used when nonzero)
    GB = np.zeros((128, 2), np.float32)
    for p in range(4):
        GB[32 * p : 32 * p + D, 0] = b1
        GB[32 * p : 32 * p + D, 1] = b2
    c["gb"] = GB
    return c


CONST_SPECS = {
    "wqk1": ((128, 128), BF16),
    "wk2": ((128, 128), BF16),
    "woe": ((128, 32), BF16),
    "cb1": ((128, 128), BF16),
    "cb2": ((128, 128), BF16),
    "selr": ((128, 128), BF16),
    "selg1": ((128, 128), BF16),
    "selg2": ((128, 128), BF16),
    "selb": ((128, 128), BF16),
    "w1e": ((128, 4 * 64), BF16),
    "w2e": ((128, 2 * 32), BF16),
    "idtb": ((32, 32), BF16),
    "gb": ((128, 2), F32),
}


def _pin_act_tables():
    """Pin Exp/Ln to natural_log_exp_and_others so LN's Ln+Exp never thrash."""
    import concourse.bacc as _bacc
    if getattr(_bacc, "_act_tables_pinned", False):
        return
    _orig = _bacc.get_activation_tables

    def _patched(arch):
        tables = dict(_orig(arch))
        keep = "natural_log_exp_and_others"
        for name in list(tables):
            if name != keep and (AF.Exp in tables[name] or AF.Ln in tables[name]):
                tables[name] = set()
        return tables

    _bacc.get_activation_tables = _patched
    _bacc._act_tables_pinned = True


def build_nc(nb: int = NB, use_b: bool = False) -> bass.Bass:
    _pin_act_tables()
    ngroups = nb // 4
    nc = bacc.Bacc()
    x_in = nc.dram_tensor("x", [nb, S, D], F32, kind="ExternalInput")
    out = nc.dram_tensor("out", [nb, S, D], F32, kind="ExternalOutput")
    cin = {
        k: nc.dram_tensor(k, list(sh), dt, kind="ExternalInput")
        for k, (sh, dt) in CONST_SPECS.items()
    }

    with tile.TileContext(nc) as tc:
        import contextlib

        ctx = contextlib.ExitStack()
        with ctx:
            constp = ctx.enter_context(tc.tile_pool(name="consts", bufs=1))
            vdp = ctx.enter_context(tc.tile_pool(name="vdp", bufs=1))
            xnp = ctx.enter_context(tc.tile_pool(name="xn", bufs=2))
            xtp = ctx.enter_context(tc.tile_pool(name="xt", bufs=2))
            xtbp = ctx.enter_context(tc.tile_pool(name="xtb", bufs=2))
            qkp = ctx.enter_context(tc.tile_pool(name="qk", bufs=2))
            ep = ctx.enter_context(tc.tile_pool(name="e", bufs=2))
            uosp = ctx.enter_context(tc.tile_pool(name="uos", bufs=2))
            rcpp = ctx.enter_context(tc.tile_pool(name="rcp", bufs=2))
            otp = ctx.enter_context(tc.tile_pool(name="ot", bufs=2))
            y1p = ctx.enter_context(tc.tile_pool(name="y1", bufs=2))
            lnp = ctx.enter_context(tc.tile_pool(name="ln", bufs=2))
            x1p = ctx.enter_context(tc.tile_pool(name="x1", bufs=2))
            hsp = ctx.enter_context(tc.tile_pool(name="hs", bufs=2))
            fsp = ctx.enter_context(tc.tile_pool(name="fs", bufs=2))
            ytp = ctx.enter_context(tc.tile_pool(name="yt", bufs=2))
            # PSUM: stt 2x[128,1024] + half 2x[128,512] + uo 2x[128,512] = 8 banks
            psA = ctx.enter_context(tc.tile_pool(name="psA", bufs=2, space="PSUM"))
            psB = ctx.enter_context(tc.tile_pool(name="psB", bufs=2, space="PSUM"))
            psC = ctx.enter_context(tc.tile_pool(name="psC", bufs=2, space="PSUM"))

            # prefetch group 0's x before the const DMAs so batch 0's
            # transpose chain overlaps the constant loads
            U0 = xnp.tile([128, 16, 32], F32, name="xu0")
            nc.gpsimd.memset(U0[:, :, D:32], 0.0)
            for pp in range(4):
                nc.sync.dma_start(
                    out=U0[32 * pp : 32 * pp + 32, :, 0:D],
                    in_=x_in[pp].rearrange("(f c) d -> c f d", c=32),
                )
            C = {}
            for k, (sh, dt) in CONST_SPECS.items():
                t = constp.tile(list(sh), dt, name=f"c_{k}")
                nc.sync.dma_start(out=t, in_=cin[k][:, :])
                C[k] = t
            eps_t = constp.tile([128, 1], F32, name="c_eps")
            nc.vector.memset(eps_t, EPS)

            # two persistent VD tiles [128, t4, hp2, i2, m128] fp8 (ping-pong).
            # ones preset at m = 32*(2hp+i) + {0, 7..31} inside each head band
            # (denominator column + finite padding rows for UO).
            vd_tiles = []
            for v in range(2):
                vt = vdp.tile([128, 4, 2, 2, 128], FP8, name=f"vd{v}")
                nc.gpsimd.memset(vt[:, :, :, :, :], 0.0)
                for hp in range(2):
                    for i in range(2):
                        m0 = 32 * (2 * hp + i)
                        nc.gpsimd.memset(vt[:, :, hp, i, m0 : m0 + 1], 1.0)
                        nc.gpsimd.memset(vt[:, :, hp, i, m0 + 7 : m0 + 32], 1.0)
                vd_tiles.append(vt)

            def ln_stages(Y, selg, bslice, OUT, outb=None):
                """LayerNorm over banded d of Y [128,512] fp32r, split into 4
                stages so PE never blocks on the DVE/ACT hops in between."""
                st = {}

                def s1():
                    mps = psB.tile([128, S], F32, name="mps", tag="half")
                    nc.tensor.matmul(mps[:, :], C["cb1"][:, :], Y[:, :],
                                     start=True, stop=True)
                    MU1 = lnp.tile([128, S], BF16, name="mu1", tag="mu")
                    nc.vector.tensor_copy(MU1[:, :], mps[:, :])
                    st["MU1"] = MU1

                def s2():
                    bcmu = psB.tile([128, S], F32, name="bcmu", tag="half")
                    nc.tensor.matmul(bcmu[:, :], C["selr"][:, :], st["MU1"][:, :],
                                     start=True, stop=True)
                    YC = lnp.tile([128, S], F32, name="yc", tag="yc")
                    nc.vector.tensor_add(YC[:, :], Y[:, :], bcmu[:, :])
                    YC2 = lnp.tile([128, S], BF16, name="yc2", tag="yc2")
                    nc.vector.tensor_mul(YC2[:, :], YC[:, :], YC[:, :])
                    st["YC"], st["YC2"] = YC, YC2

                def s3():
                    vps = psB.tile([128, S], F32, name="vps", tag="half")
                    nc.tensor.matmul(vps[:, :], C["cb2"][:, :], st["YC2"][:, :],
                                     start=True, stop=True)
                    LNV = lnp.tile([128, S], F32, name="lnv", tag="lnv")
                    nc.scalar.activation(LNV[:, :], vps[:, :], AF.Ln,
                                         bias=eps_t[:, :])
                    RS1 = lnp.tile([128, S], BF16, name="rs1", tag="rs")
                    nc.scalar.activation(RS1[:, :], LNV[:, :], AF.Exp, scale=-0.5)
                    st["RS1"] = RS1

                def s4():
                    bcrs = psB.tile([128, S], F32, name="bcrs", tag="half")
                    nc.tensor.matmul(bcrs[:, :], selg[:, :], st["RS1"][:, :],
                                     start=True, stop=True)
                    if use_b:
                        TMP = lnp.tile([128, S], F32, name="lntmp", tag="tmp")
                        nc.vector.tensor_mul(TMP[:, :], st["YC"][:, :], bcrs[:, :])
                        nc.vector.tensor_scalar(out=OUT[:, :], in0=TMP[:, :],
                                                scalar1=bslice, scalar2=None,
                                                op0=ALU.add)
                    else:
                        nc.vector.tensor_mul(OUT[:, :], st["YC"][:, :], bcrs[:, :])
                    if outb is not None:
                        nc.gpsimd.tensor_copy(outb[:, :], OUT[:, :])

                return [s1, s2, s3, s4]

            def batch_head(g, p, XT4b, filler):
                """QKV + scores + E-gen for batch 4g+p. Returns state."""
                ps1 = psB.tile([128, S], F32, name="ps1", tag="half")
                nc.tensor.matmul(
                    ps1[:, :], C["wqk1"][32 * p : 32 * p + D, :],
                    XT4b[32 * p : 32 * p + D, :],
                    start=True, stop=True, tile_position=(32 * p, 0),
                )
                ps2 = psB.tile([128, S], F32, name="ps2", tag="half")
                nc.tensor.matmul(
                    ps2[:, :], C["wk2"][32 * p : 32 * p + D, :],
                    XT4b[32 * p : 32 * p + D, :],
                    start=True, stop=True, tile_position=(32 * p, 0),
                )
                QK = qkp.tile([128, 2 * S], BF16, name="qk")
                nc.vector.tensor_copy(QK[:, 0:S], ps1[:, :])
                nc.scalar.activation(QK[:, S : 2 * S], ps2[:, :], AF.Copy)

                # V^T via PE transposes of rows 0:32 (V lives at rows 8:32)
                psv = psB.tile([128, 4 * 32], BF16, name="psv", tag="half")
                for t in range(4):
                    nc.tensor.transpose(
                        psv[:, 32 * t : 32 * (t + 1)],
                        QK[0:32, 128 * t : 128 * (t + 1)],
                        C["idtb"][:, :],
                    )
                VD = vd_tiles[(4 * g + p) % 2]
                # VD[k, t, hp, i, 32*(2hp+i)+1 : +7] = psv[k, 32t+8+6(2hp+i) : +6]
                vd_dst = bass.AP(
                    tensor=VD.tensor, offset=VD.offset + 1,
                    ap=[list(VD.ap[0]), [512, 4], [320, 2], [160, 2], [1, 6]],
                )
                vd_src = bass.AP(
                    tensor=psv.tensor, offset=psv.offset + 8,
                    ap=[list(psv.ap[0]), [32, 4], [12, 2], [6, 2], [1, 6]],
                )
                nc.vector.tensor_copy(vd_dst, vd_src)

                # scores (S^T, raw) + E-gen (fp8e4m3 bits via affine).
                # After each score pair, pop one filler closure (AV of the
                # previous batch / tail matmuls / group stages) so the PE
                # queue always has ready work between stt slot waits.
                E = ep.tile([128, 4, 2, 2, S], FP8, name="e")
                for t in range(4):
                    for hp in range(2):
                        stt = psA.tile([128, 2 * S], F32, name="stt", tag="big")
                        for i in range(2):
                            h = 2 * hp + i
                            nc.tensor.matmul(
                                stt[:, 512 * i : 512 * (i + 1)],
                                QK[32 * h : 32 * h + HD,
                                   S + 128 * t : S + 128 * (t + 1)],
                                QK[32 * h : 32 * h + HD, 0:S],
                                start=True, stop=True,
                                tile_position=(32 * h, 0),
                                skip_group_check=True,
                            )
                        eng = EGEN_ENGINES[2 * t + hp]
                        e_out = E[:, t, hp, :, :].bitcast(U8)
                        s_in = stt.rearrange("p (i q) -> p i q", i=2)
                        if eng == "act":
                            nc.scalar.activation(e_out, s_in, AF.Copy,
                                                 bias=EB, scale=EA)
                        else:
                            nc.vector.tensor_scalar(
                                out=e_out, in0=s_in, scalar1=EA, scalar2=EB,
                                op0=ALU.mult, op1=ALU.add,
                            )
                        if filler:
                            filler.pop(0)()

                return {"E": E, "VD": VD, "p": p}

            def av_closures(state):
                """fp8 DoubleRow AV as 8 single-matmul closures (interleavable)."""
                E, VD = state["E"], state["VD"]
                UO = psC.tile([128, S], F32, name="uo", tag="uo")
                state["UO"] = UO

                def mk(t, hp, idx):
                    def go():
                        nc.tensor.matmul(
                            UO[:, :], VD[:, t, hp, :, :], E[:, t, hp, :, :],
                            start=(idx == 0), stop=(idx == 7),
                            perf_mode=DR, tile_position=(0, 0),
                            skip_group_check=True,
                        )
                    return go

                return [mk(t, hp, 2 * t + hp) for t in range(4) for hp in range(2)]

            def tail_closures(state, XT4, Y1):
                """normalize + Wo + residual for a batch, as two closures."""
                p = state["p"]
                hold = {}

                def t1():
                    UOs = uosp.tile([128, S], BF16, name="uos")
                    nc.scalar.activation(UOs[:, :], state["UO"][:, :], AF.Copy)
                    denps = psB.tile([128, S], F32, name="denps", tag="half")
                    nc.tensor.matmul(denps[:, :], C["selb"][:, :], UOs[:, :],
                                     start=True, stop=True)
                    hold["UOs"], hold["denps"] = UOs, denps

                def t2a():
                    RCP = rcpp.tile([128, S], F32, name="rcpt")
                    nc.vector.reciprocal_approx_fast(RCP[:, :], hold["denps"][:, :])
                    OTn = otp.tile([128, S], BF16, name="otn")
                    nc.vector.tensor_mul(OTn[:, :], hold["UOs"][:, :], RCP[:, :])
                    hold["OTn"] = OTn

                def t2b():
                    wops = psB.tile([32, S], F32, name="wops", tag="half")
                    nc.tensor.matmul(wops[:, :], C["woe"][:, :], hold["OTn"][:, :],
                                     start=True, stop=True)
                    nc.vector.tensor_add(
                        Y1[32 * p : 32 * p + 32, :], wops[:, :],
                        XT4[32 * p : 32 * p + 32, :],
                    )

                return [t1, t2a, t2b]

            def group_stages(g, Y1):
                """LN1 + FFN + LN2 + output store for group g as a stage list."""
                X1 = x1p.tile([128, S], BF16, name="x1")
                stages = ln_stages(Y1, C["selg1"], C["gb"][:, 0:1], X1)
                st = {}

                def ffn_a():
                    F4 = psC.tile([128, S], F32, name="f4", tag="uo")
                    hps = psB.tile([128, S], F32, name="hps", tag="half")
                    for j in range(2):
                        nc.tensor.matmul(
                            hps[64 * j : 64 * j + 64, :],
                            C["w1e"][:, 64 * j : 64 * (j + 1)],
                            X1[:, :],
                            start=True, stop=True, tile_position=(0, 64 * j),
                            skip_group_check=True,
                        )
                    HS = hsp.tile([128, S], BF16, name="hs")
                    nc.scalar.activation(HS[:, :], hps[:, :], AF.Relu)
                    st["F4"], st["HS"] = F4, HS

                def ffn_b():
                    F4 = st["F4"]
                    for j in range(2):
                        nc.tensor.matmul(
                            F4[32 * j : 32 * j + 32, :],
                            C["w2e"][:, 32 * j : 32 * (j + 1)],
                            st["HS"][:, :],
                            start=True, stop=True, tile_position=(0, 32 * j),
                            skip_group_check=True,
                        )
                    hps = psB.tile([128, S], F32, name="hps2", tag="half")
                    for j in range(2):
                        nc.tensor.matmul(
                            hps[64 * j : 64 * j + 64, :],
                            C["w1e"][:, 64 * (2 + j) : 64 * (3 + j)],
                            X1[:, :],
                            start=True, stop=True, tile_position=(0, 64 * j),
                            skip_group_check=True,
                        )
                    HS2 = hsp.tile([128, S], BF16, name="hs2", tag="hs2")
                    nc.scalar.activation(HS2[:, :], hps[:, :], AF.Relu)
                    st["HS2"] = HS2

                def ffn_c():
                    F4 = st["F4"]
                    for j in range(2):
                        nc.tensor.matmul(
                            F4[32 * (2 + j) : 32 * (3 + j), :],
                            C["w2e"][:, 32 * j : 32 * (j + 1)],
                            st["HS2"][:, :],
                            start=True, stop=True, tile_position=(0, 32 * (2 + j)),
                            skip_group_check=True,
                        )
                    FS0 = fsp.tile([128, S], BF16, name="fs0", tag="fs0")
                    nc.scalar.activation(FS0[:, :], F4[:, :], AF.Relu)
                    FS = fsp.tile([128, S], BF16, name="fst", tag="fst")
                    nc.gpsimd.tensor_add(FS[:, :], FS0[:, :], X1[:, :])
                    st["FS"] = FS

                stages += [ffn_a, ffn_b, ffn_c]

                Y2N = ytp.tile([128, S], F32, name="y2n", tag="y2n")
                ln2 = [None]

                def ln2_s1():
                    ln2[0] = ln_stages(st["FS"], C["selg2"], C["gb"][:, 1:2], Y2N)
                    ln2[0][0]()

                def emit_out():
                    Y2T = ytp.tile([128, S], F32, name="y2t", tag="y2t")
                    nc.vector.transpose(Y2T[:, :], Y2N[:, :])
                    for p in range(4):
                        b = 4 * g + p
                        nc.sync.dma_start(
                            out=out[b].rearrange("(f r) d -> r f d", r=32),
                            in_=Y2T[32 * p : 32 * p + 32, :].rearrange(
                                "r (f c) -> r f c", c=32
                            )[:, :, 0:D],
                        )

                stages += [ln2_s1,
                           lambda: ln2[0][1](),
                           lambda: ln2[0][2](),
                           lambda: ln2[0][3](),
                           emit_out]
                return stages

            # slot machine: headA(i) | AV(i-1) | tail(i-2) | <=3 group stages
            states = {}
            gctx = {}
            pending = []
            nslots = 4 * ngroups

            def emit_slot(i):
                g, p = divmod(i, 4)
                if p == 0:
                    if g == 0:
                        U = U0
                    else:
                        U = xnp.tile([128, 16, 32], F32, name="xu")
                        nc.gpsimd.memset(U[:, :, D:32], 0.0)
                        for pp in range(4):
                            nc.sync.dma_start(
                                out=U[32 * pp : 32 * pp + 32, :, 0:D],
                                in_=x_in[4 * g + pp].rearrange("(f c) d -> c f d", c=32),
                            )
                    XT4 = xtp.tile([128, S], F32, name="xt4")
                    nc.vector.transpose(XT4[:, :], U.rearrange("p a b -> p (a b)"))
                    XT4b = xtbp.tile([128, S], BF16, name="xt4b")
                    nc.vector.tensor_copy(XT4b[:, :], XT4[:, :])
                    Y1 = y1p.tile([128, S], BF16, name="y1")
                    gctx[g] = (XT4, XT4b, Y1)
                XT4, XT4b, Y1 = gctx[g]
                base = []
                post = []
                gpush = None
                if i - 1 in states:
                    base += av_closures(states[i - 1])
                if i - 2 in states:
                    s = states.pop(i - 2)
                    g2 = (i - 2) // 4
                    tcs = tail_closures(s, gctx[g2][0], gctx[g2][2])
                    base += tcs[:2]
                    post.append(tcs[2])
                    if (i - 2) % 4 == 3:
                        gpush = g2   # push AFTER t2b writes Y1's last band
                gs = []
                for _ in range(3):
                    if pending:
                        gs.append(pending.pop(0))
                # weave group stages ~4 filler positions apart so their
                # dependency chains clear before the next stage's PE matmul
                filler = (base[0:3] + gs[0:1] + base[3:6] + gs[1:2]
                          + base[6:9] + gs[2:3] + base[9:])
                states[i] = batch_head(g, p, XT4b, filler)
                while filler:
                    filler.pop(0)()
                for f in post:
                    f()
                if gpush is not None:
                    pending.extend(group_stages(gpush, gctx[gpush][2]))

            for i in range(nslots):
                emit_slot(i)
            # drain: interleave the leftover stages of the second-to-last
            # group with the last group's stages so one group's matmuls fill
            # the other's ACT/DVE chain latency
            for f in av_closures(states[nslots - 1]):
                f()
            for i in (nslots - 2, nslots - 1):
                s = states.pop(i)
                g2 = i // 4
                for f in tail_closures(s, gctx[g2][0], gctx[g2][2]):
                    f()
            pre = list(pending)
            last = group_stages(ngroups - 1, gctx[ngroups - 1][2])
            pending.clear()
            while pre or last:
                if last:
                    last.pop(0)()
                if pre:
                    pre.pop(0)()

    nc.compile()
    return nc


def build_nc(nb: int = NB, use_b: bool = False) -> bass.Bass:
    _pin_act_tables()
    ngroups = nb // 4
    nc = bacc.Bacc()
    x_in = nc.dram_tensor("x", [nb, S, D], F32, kind="ExternalInput")
    out = nc.dram_tensor("out", [nb, S, D], F32, kind="ExternalOutput")
    cin = {
        k: nc.dram_tensor(k, list(sh), dt, kind="ExternalInput")
        for k, (sh, dt) in CONST_SPECS.items()
    }

    with tile.TileContext(nc) as tc:
        import contextlib

        ctx = contextlib.ExitStack()
        with ctx:
            constp = ctx.enter_context(tc.tile_pool(name="consts", bufs=1))
            vdp = ctx.enter_context(tc.tile_pool(name="vdp", bufs=1))
            xnp = ctx.enter_context(tc.tile_pool(name="xn", bufs=2))
            xtp = ctx.enter_context(tc.tile_pool(name="xt", bufs=2))
            xtbp = ctx.enter_context(tc.tile_pool(name="xtb", bufs=2))
            qkp = ctx.enter_context(tc.tile_pool(name="qk", bufs=2))
            ep = ctx.enter_context(tc.tile_pool(name="e", bufs=2))
            uosp = ctx.enter_context(tc.tile_pool(name="uos", bufs=2))
            rcpp = ctx.enter_context(tc.tile_pool(name="rcp", bufs=2))
            otp = ctx.enter_context(tc.tile_pool(name="ot", bufs=2))
            y1p = ctx.enter_context(tc.tile_pool(name="y1", bufs=2))
            lnp = ctx.enter_context(tc.tile_pool(name="ln", bufs=2))
            x1p = ctx.enter_context(tc.tile_pool(name="x1", bufs=2))
            hsp = ctx.enter_context(tc.tile_pool(name="hs", bufs=2))
            fsp = ctx.enter_context(tc.tile_pool(name="fs", bufs=2))
            ytp = ctx.enter_context(tc.tile_pool(name="yt", bufs=2))
            # PSUM: stt 2x[128,1024] + half 2x[128,512] + uo 2x[128,512] = 8 banks
            psA = ctx.enter_context(tc.tile_pool(name="psA", bufs=2, space="PSUM"))
            psB = ctx.enter_context(tc.tile_pool(name="psB", bufs=2, space="PSUM"))
            psC = ctx.enter_context(tc.tile_pool(name="psC", bufs=2, space="PSUM"))

            # prefetch group 0's x before the const DMAs so batch 0's
            # transpose chain overlaps the constant loads
            U0 = xnp.tile([128, 16, 32], F32, name="xu0")
            nc.gpsimd.memset(U0[:, :, D:32], 0.0)
            for pp in range(4):
                nc.sync.dma_start(
                    out=U0[32 * pp : 32 * pp + 32, :, 0:D],
                    in_=x_in[pp].rearrange("(f c) d -> c f d", c=32),
                )
            C = {}
            for k, (sh, dt) in CONST_SPECS.items():
                t = constp.tile(list(sh), dt, name=f"c_{k}")
                nc.sync.dma_start(out=t, in_=cin[k][:, :])
                C[k] = t
            eps_t = constp.tile([128, 1], F32, name="c_eps")
            nc.vector.memset(eps_t, EPS)

            # two persistent VD tiles [128, t4, hp2, i2, m128] fp8 (ping-pong).
            # ones preset at m = 32*(2hp+i) + {0, 7..31} inside each head band
            # (denominator column + finite padding rows for UO).
            vd_tiles = []
            for v in range(2):
                vt = vdp.tile([128, 4, 2, 2, 128], FP8, name=f"vd{v}")
                nc.gpsimd.memset(vt[:, :, :, :, :], 0.0)
                for hp in range(2):
                    for i in range(2):
                        m0 = 32 * (2 * hp + i)
                        nc.gpsimd.memset(vt[:, :, hp, i, m0 : m0 + 1], 1.0)
                        nc.gpsimd.memset(vt[:, :, hp, i, m0 + 7 : m0 + 32], 1.0)
                vd_tiles.append(vt)

            def ln_stages(Y, selg, bslice, OUT, outb=None):
                """LayerNorm over banded d of Y [128,512] fp32r, split into 4
                stages so PE never blocks on the DVE/ACT hops in between."""
                st = {}

                def s1():
                    mps = psB.tile([128, S], F32, name="mps", tag="half")
                    nc.tensor.matmul(mps[:, :], C["cb1"][:, :], Y[:, :],
                                     start=True, stop=True)
                    MU1 = lnp.tile([128, S], BF16, name="mu1", tag="mu")
                    nc.vector.tensor_copy(MU1[:, :], mps[:, :])
                    st["MU1"] = MU1

                def s2():
                    bcmu = psB.tile([128, S], F32, name="bcmu", tag="half")
                    nc.tensor.matmul(bcmu[:, :], C["selr"][:, :], st["MU1"][:, :],
                                     start=True, stop=True)
                    YC = lnp.tile([128, S], F32, name="yc", tag="yc")
                    nc.vector.tensor_add(YC[:, :], Y[:, :], bcmu[:, :])
                    YC2 = lnp.tile([128, S], BF16, name="yc2", tag="yc2")
                    nc.vector.tensor_mul(YC2[:, :], YC[:, :], YC[:, :])
                    st["YC"], st["YC2"] = YC, YC2

                def s3():
                    vps = psB.tile([128, S], F32, name="vps", tag="half")
                    nc.tensor.matmul(vps[:, :], C["cb2"][:, :], st["YC2"][:, :],
                                     start=True, stop=True)
                    LNV = lnp.tile([128, S], F32, name="lnv", tag="lnv")
                    nc.scalar.activation(LNV[:, :], vps[:, :], AF.Ln,
                                         bias=eps_t[:, :])
                    RS1 = lnp.tile([128, S], BF16, name="rs1", tag="rs")
                    nc.scalar.activation(RS1[:, :], LNV[:, :], AF.Exp, scale=-0.5)
                    st["RS1"] = RS1

                def s4():
                    bcrs = psB.tile([128, S], F32, name="bcrs", tag="half")
                    nc.tensor.matmul(bcrs[:, :], selg[:, :], st["RS1"][:, :],
                                     start=True, stop=True)
                    if use_b:
                        TMP = lnp.tile([128, S], F32, name="lntmp", tag="tmp")
                        nc.vector.tensor_mul(TMP[:, :], st["YC"][:, :], bcrs[:, :])
                        nc.vector.tensor_scalar(out=OUT[:, :], in0=TMP[:, :],
                                                scalar1=bslice, scalar2=None,
                                                op0=ALU.add)
                    else:
                        nc.vector.tensor_mul(OUT[:, :], st["YC"][:, :], bcrs[:, :])
                    if outb is not None:
                        nc.gpsimd.tensor_copy(outb[:, :], OUT[:, :])

                return [s1, s2, s3, s4]

            def batch_head(g, p, XT4b, filler):
                """QKV + scores + E-gen for batch 4g+p. Returns state."""
                ps1 = psB.tile([128, S], F32, name="ps1", tag="half")
                nc.tensor.matmul(
                    ps1[:, :], C["wqk1"][32 * p : 32 * p + D, :],
                    XT4b[32 * p : 32 * p + D, :],
                    start=True, stop=True, tile_position=(32 * p, 0),
                )
                ps2 = psB.tile([128, S], F32, name="ps2", tag="half")
                nc.tensor.matmul(
                    ps2[:, :], C["wk2"][32 * p : 32 * p + D, :],
                    XT4b[32 * p : 32 * p + D, :],
                    start=True, stop=True, tile_position=(32 * p, 0),
                )
                QK = qkp.tile([128, 2 * S], BF16, name="qk")
                nc.vector.tensor_copy(QK[:, 0:S], ps1[:, :])
                nc.scalar.activation(QK[:, S : 2 * S], ps2[:, :], AF.Copy)

                # V^T via PE transposes of rows 0:32 (V lives at rows 8:32)
                psv = psB.tile([128, 4 * 32], BF16, name="psv", tag="half")
                for t in range(4):
                    nc.tensor.transpose(
                        psv[:, 32 * t : 32 * (t + 1)],
                        QK[0:32, 128 * t : 128 * (t + 1)],
                        C["idtb"][:, :],
                    )
                VD = vd_tiles[(4 * g + p) % 2]
                # VD[k, t, hp, i, 32*(2hp+i)+1 : +7] = psv[k, 32t+8+6(2hp+i) : +6]
                vd_dst = bass.AP(
                    tensor=VD.tensor, offset=VD.offset + 1,
                    ap=[list(VD.ap[0]), [512, 4], [320, 2], [160, 2], [1, 6]],
                )
                vd_src = bass.AP(
                    tensor=psv.tensor, offset=psv.offset + 8,
                    ap=[list(psv.ap[0]), [32, 4], [12, 2], [6, 2], [1, 6]],
                )
                nc.vector.tensor_copy(vd_dst, vd_src)

                # scores (S^T, raw) + E-gen (fp8e4m3 bits via affine).
                # After each score pair, pop one filler closure (AV of the
                # previous batch / tail matmuls / group stages) so the PE
                # queue always has ready work between stt slot waits.
                E = ep.tile([128, 4, 2, 2, S], FP8, name="e")
                for t in range(4):
                    for hp in range(2):
                        stt = psA.tile([128, 2 * S], F32, name="stt", tag="big")
                        for i in range(2):
                            h = 2 * hp + i
                            nc.tensor.matmul(
                                stt[:, 512 * i : 512 * (i + 1)],
                                QK[32 * h : 32 * h + HD,
                                   S + 128 * t : S + 128 * (t + 1)],
                                QK[32 * h : 32 * h + HD, 0:S],
                                start=True, stop=True,
                                tile_position=(32 * h, 0),
                                skip_group_check=True,
                            )
                        eng = EGEN_ENGINES[2 * t + hp]
                        e_out = E[:, t, hp, :, :].bitcast(U8)
                        s_in = stt.rearrange("p (i q) -> p i q", i=2)
                        if eng == "act":
                            nc.scalar.activation(e_out, s_in, AF.Copy,
                                                 bias=EB, scale=EA)
                        else:
                            nc.vector.tensor_scalar(
                                out=e_out, in0=s_in, scalar1=EA, scalar2=EB,
                                op0=ALU.mult, op1=ALU.add,
                            )
                        if filler:
                            filler.pop(0)()

                return {"E": E, "VD": VD, "p": p}

            def av_closures(state):
                """fp8 DoubleRow AV as 8 single-matmul closures (interleavable)."""
                E, VD = state["E"], state["VD"]
                UO = psC.tile([128, S], F32, name="uo", tag="uo")
                state["UO"] = UO

                def mk(t, hp, idx):
                    def go():
                        nc.tensor.matmul(
                            UO[:, :], VD[:, t, hp, :, :], E[:, t, hp, :, :],
                            start=(idx == 0), stop=(idx == 7),
                            perf_mode=DR, tile_position=(0, 0),
                            skip_group_check=True,
                        )
                    return go

                return [mk(t, hp, 2 * t + hp) for t in range(4) for hp in range(2)]

            def tail_closures(state, XT4, Y1):
                """normalize + Wo + residual for a batch, as two closures."""
                p = state["p"]
                hold = {}

                def t1():
                    UOs = uosp.tile([128, S], BF16, name="uos")
                    nc.scalar.activation(UOs[:, :], state["UO"][:, :], AF.Copy)
                    denps = psB.tile([128, S], F32, name="denps", tag="half")
                    nc.tensor.matmul(denps[:, :], C["selb"][:, :], UOs[:, :],
                                     start=True, stop=True)
                    hold["UOs"], hold["denps"] = UOs, denps

                def t2a():
                    RCP = rcpp.tile([128, S], F32, name="rcpt")
                    nc.vector.reciprocal_approx_fast(RCP[:, :], hold["denps"][:, :])
                    OTn = otp.tile([128, S], BF16, name="otn")
                    nc.vector.tensor_mul(OTn[:, :], hold["UOs"][:, :], RCP[:, :])
                    hold["OTn"] = OTn

                def t2b():
                    wops = psB.tile([32, S], F32, name="wops", tag="half")
                    nc.tensor.matmul(wops[:, :], C["woe"][:, :], hold["OTn"][:, :],
                                     start=True, stop=True)
                    nc.vector.tensor_add(
                        Y1[32 * p : 32 * p + 32, :], wops[:, :],
                        XT4[32 * p : 32 * p + 32, :],
                    )

                return [t1, t2a, t2b]

            def group_stages(g, Y1):
                """LN1 + FFN + LN2 + output store for group g as a stage list."""
                X1 = x1p.tile([128, S], BF16, name="x1")
                stages = ln_stages(Y1, C["selg1"], C["gb"][:, 0:1], X1)
                st = {}

                def ffn_a():
                    F4 = psC.tile([128, S], F32, name="f4", tag="uo")
                    hps = psB.tile([128, S], F32, name="hps", tag="half")
                    for j in range(2):
                        nc.tensor.matmul(
                            hps[64 * j : 64 * j + 64, :],
                            C["w1e"][:, 64 * j : 64 * (j + 1)],
                            X1[:, :],
                            start=True, stop=True, tile_position=(0, 64 * j),
                            skip_group_check=True,
                        )
                    HS = hsp.tile([128, S], BF16, name="hs")
                    nc.scalar.activation(HS[:, :], hps[:, :], AF.Relu)
                    st["F4"], st["HS"] = F4, HS

                def ffn_b():
                    F4 = st["F4"]
                    for j in range(2):
                        nc.tensor.matmul(
                            F4[32 * j : 32 * j + 32, :],
                            C["w2e"][:, 32 * j : 32 * (j + 1)],
                            st["HS"][:, :],
                            start=True, stop=True, tile_position=(0, 32 * j),
                            skip_group_check=True,
                        )
                    hps = psB.tile([128, S], F32, name="hps2", tag="half")
                    for j in range(2):
                        nc.tensor.matmul(
                            hps[64 * j : 64 * j + 64, :],
                            C["w1e"][:, 64 * (2 + j) : 64 * (3 + j)],
                            X1[:, :],
                            start=True, stop=True, tile_position=(0, 64 * j),
                            skip_group_check=True,
                        )
                    HS2 = hsp.tile([128, S], BF16, name="hs2", tag="hs2")
                    nc.scalar.activation(HS2[:, :], hps[:, :], AF.Relu)
                    st["HS2"] = HS2

                def ffn_c():
                    F4 = st["F4"]
                    for j in range(2):
                        nc.tensor.matmul(
                            F4[32 * (2 + j) : 32 * (3 + j), :],
                            C["w2e"][:, 32 * j : 32 * (j + 1)],
                            st["HS2"][:, :],
                            start=True, stop=True, tile_position=(0, 32 * (2 + j)),
                            skip_group_check=True,
                        )
                    FS0 = fsp.tile([128, S], BF16, name="fs0", tag="fs0")
                    nc.scalar.activation(FS0[:, :], F4[:, :], AF.Relu)
                    FS = fsp.tile([128, S], BF16, name="fst", tag="fst")
                    nc.gpsimd.tensor_add(FS[:, :], FS0[:, :], X1[:, :])
                    st["FS"] = FS

                stages += [ffn_a, ffn_b, ffn_c]

                Y2N = ytp.tile([128, S], F32, name="y2n", tag="y2n")
                ln2 = [None]

                def ln2_s1():
                    ln2[0] = ln_stages(st["FS"], C["selg2"], C["gb"][:, 1:2], Y2N)
                    ln2[0][0]()

                def emit_out():
                    Y2T = ytp.tile([128, S], F32, name="y2t", tag="y2t")
                    nc.vector.transpose(Y2T[:, :], Y2N[:, :])
                    for p in range(4):
                        b = 4 * g + p
                        nc.sync.dma_start(
                            out=out[b].rearrange("(f r) d -> r f d", r=32),
                            in_=Y2T[32 * p : 32 * p + 32, :].rearrange(
                                "r (f c) -> r f c", c=32
                            )[:, :, 0:D],
                        )

                stages += [ln2_s1,
                           lambda: ln2[0][1](),
                           lambda: ln2[0][2](),
                           lambda: ln2[0][3](),
                           emit_out]
                return stages

            # slot machine: headA(i) | AV(i-1) | tail(i-2) | <=3 group stages
            states = {}
            gctx = {}
            pending = []
            nslots = 4 * ngroups

            def emit_slot(i):
                g, p = divmod(i, 4)
                if p == 0:
                    if g == 0:
                        U = U0
                    else:
                        U = xnp.tile([128, 16, 32], F32, name="xu")
                        nc.gpsimd.memset(U[:, :, D:32], 0.0)
                        for pp in range(4):
                            nc.sync.dma_start(
                                out=U[32 * pp : 32 * pp + 32, :, 0:D],
                                in_=x_in[4 * g + pp].rearrange("(f c) d -> c f d", c=32),
                            )
                    XT4 = xtp.tile([128, S], F32, name="xt4")
                    nc.vector.transpose(XT4[:, :], U.rearrange("p a b -> p (a b)"))
                    XT4b = xtbp.tile([128, S], BF16, name="xt4b")
                    nc.vector.tensor_copy(XT4b[:, :], XT4[:, :])
                    Y1 = y1p.tile([128, S], BF16, name="y1")
                    gctx[g] = (XT4, XT4b, Y1)
                XT4, XT4b, Y1 = gctx[g]
                base = []
                post = []
                gpush = None
                if i - 1 in states:
                    base += av_closures(states[i - 1])
                if i - 2 in states:
                    s = states.pop(i - 2)
                    g2 = (i - 2) // 4
                    tcs = tail_closures(s, gctx[g2][0], gctx[g2][2])
                    base += tcs[:2]
                    post.append(tcs[2])
                    if (i - 2) % 4 == 3:
                        gpush = g2   # push AFTER t2b writes Y1's last band
                gs = []
                for _ in range(3):
                    if pending:
                        gs.append(pending.pop(0))
                # weave group stages ~4 filler positions apart so their
                # dependency chains clear before the next stage's PE matmul
                filler = (base[0:3] + gs[0:1] + base[3:6] + gs[1:2]
                          + base[6:9] + gs[2:3] + base[9:])
                states[i] = batch_head(g, p, XT4b, filler)
                while filler:
                    filler.pop(0)()
                for f in post:
                    f()
                if gpush is not None:
                    pending.extend(group_stages(gpush, gctx[gpush][2]))

            for i in range(nslots):
                emit_slot(i)
            # drain: interleave the leftover stages of the second-to-last
            # group with the last group's stages so one group's matmuls fill
            # the other's ACT/DVE chain latency
            for f in av_closures(states[nslots - 1]):
                f()
            for i in (nslots - 2, nslots - 1):
                s = states.pop(i)
                g2 = i // 4
                for f in tail_closures(s, gctx[g2][0], gctx[g2][2]):
                    f()
            pre = list(pending)
            last = group_stages(ngroups - 1, gctx[ngroups - 1][2])
            pending.clear()
            while pre or last:
                if last:
                    last.pop(0)()
                if pre:
                    pre.pop(0)()

    nc.compile()
    return nc


def _enable_ldw_opt():
    """Flip walrus --enable-ldw-opt to true (hides LDWEIGHTS behind matmuls)."""
    import concourse.bass_utils as _bu
    if getattr(_bu, "_ldw_opt_patched", False):
        return
    _orig = _bu.run_command

    def _patched(cmd, *a, **kw):
        if isinstance(cmd, list):
            cmd = ["--enable-ldw-opt=true" if c == "--enable-ldw-opt=false" else c
                   for c in cmd]
        return _orig(cmd, *a, **kw)

    _bu.run_command = _patched
    _bu._ldw_opt_patched = True


_NC_CACHE: dict[tuple, bass.Bass] = {}


def _get_nc(nb: int, use_b: bool = False) -> bass.Bass:
    key = (nb, use_b)
    if key not in _NC_CACHE:
        _NC_CACHE[key] = build_nc(nb, use_b)
    return _NC_CACHE[key]


def kernel(x, Wq, Wk, Wv, Wo, W1, W2, g1, b1, g2, b2):
    x = np.asarray(x, np.float32)
    args = [np.asarray(a, np.float32) for a in (Wq, Wk, Wv, Wo, W1, W2, g1, b1, g2, b2)]
    consts = _host_consts(*args)
    use_b = bool(np.any(args[7]) or np.any(args[9]))
    nc = _get_nc(NB, use_b)
    in_maps = []
    for c in range(NCORES):
        m = {"x": np.ascontiguousarray(x[c * NB : (c + 1) * NB])}
        m.update(consts)
        in_maps.append(m)
    res = run_bass_kernel_spmd(nc, in_maps, list(range(NCORES)))
    return np.concatenate([r["out"] for r in res.results], axis=0)


# revision 24
# speedup vs baseline: 1.2512x; 1.0005x over previous
"""Trainium2 Bass kernel for nn_Encoder_block (B=128,S=512,D=24,H=4,HD=6,DFF=48).

Data parallel over batch: 16 batches/core x 8 cores. Per core, T-layout
([d, token], d on partitions) with 4 batches banded per 128 partitions.

v2 speedups over the 478us baseline:
  - bf16 QKV/Wo matmuls and fp32r LN/FFN matmuls (1 cyc/row instead of 4).
  - softmax exp is a single Schraudolph-style affine per score pair: fp8e4m3
    BITS = round(s_raw * 8*log2e/sqrt(6) + 55.66) computed by one
    tensor_scalar/activation into a uint8-bitcast of the E tile. Replaces
    exact ACT exp + separate fp8 quantize.
  - AV uses fp8 DoubleRow matmuls whose two planes carry two HEADS via
    block-diagonal V weights: 8 matmuls x 512 cols x 0.5 cyc covers the whole
    attention-value product, landing directly in banded T-layout with the
    softmax denominators riding along as ones-columns.
  - softmax normalize: ACT copy UO->bf16, PE selector-broadcast of the
    denominator row, DVE reciprocal + multiply (no DMA broadcasts).
  - LayerNorm: selector matmuls for mean/var, PE broadcast of -mu and
    g*rstd (g folded into the selector weights), everything fp32r.
  - per-batch "tail" (normalize+Wo) emitted one batch late so PE stays busy.
"""

import os
import sys

import numpy as np

for _p in ("/opt/trn_rl_repo", "/opt/trn_rl_repo/concourse"):
    if os.path.isdir(_p) and _p not in sys.path:
        sys.path.insert(0, _p)

import concourse.bass as bass
import concourse.bacc as bacc
import concourse.mybir as mybir
import concourse.tile as tile
from concourse.bass_utils import run_bass_kernel_spmd

F32 = mybir.dt.float32
F32R = mybir.dt.float32r
BF16 = mybir.dt.bfloat16
FP8 = mybir.dt.float8e4
U8 = mybir.dt.uint8
AF = mybir.ActivationFunctionType
ALU = mybir.AluOpType
DR = mybir.MatmulPerfMode.DoubleRow

B, S, D = 128, 512, 24
H, HD, DFF = 4, 6, 48
EPS = 1e-5
NCORES = 8
NB = B // NCORES          # batches per core = 16
SCALE = 1.0 / np.sqrt(HD)
EA = float(8.0 * np.log2(np.e) * SCALE)   # fp8e4m3 bits slope
EB = 55.66                                # fp8e4m3 bits offset (calibrated)

# E-gen engine per (t, hp) slot: balance ACT vs DVE load
EGEN_ENGINES = ["act", "dve", "act", "dve", "act", "dve", "act", "act"]


def _host_consts(Wq, Wk, Wv, Wo, W1, W2, g1, b1, g2, b2):
    import ml_dtypes
    c = {}
    # QKV lhsT (bf16): per band p: col 32h+j = Wq[6h+j,:], cols 8:32 = Wv rows
    wqk1 = np.zeros((D, 128), np.float32)
    wk2 = np.zeros((D, 128), np.float32)
    for h in range(H):
        for j in range(HD):
            wqk1[:, 32 * h + j] = Wq[6 * h + j, :]
            wk2[:, 32 * h + j] = Wk[6 * h + j, :]
    for dv in range(D):
        wqk1[:, 8 + dv] = Wv[dv, :]
    WQK1 = np.zeros((128, 128), np.float32)
    WK2 = np.zeros((128, 128), np.float32)
    for p in range(4):
        WQK1[32 * p : 32 * p + D, :] = wqk1
        WK2[32 * p : 32 * p + D, :] = wk2
    c["wqk1"] = WQK1.astype(ml_dtypes.bfloat16)
    c["wk2"] = WK2.astype(ml_dtypes.bfloat16)

    # Wo lhsT bf16: rows 32h+1+j = Wo[:, 6h+j] (row 32h is the denominator)
    WOE = np.zeros((128, 32), np.float32)
    for h in range(H):
        for j in range(HD):
            WOE[32 * h + 1 + j, 0:D] = Wo[:, 6 * h + j]
    c["woe"] = WOE.astype(ml_dtypes.bfloat16)

    # LN selectors (fp32r): cb1 col 32p = -1/24 over band p; cb2 = +1/24
    CB1 = np.zeros((128, 128), np.float32)
    CB2 = np.zeros((128, 128), np.float32)
    for p in range(4):
        CB1[32 * p : 32 * p + D, 32 * p] = -1.0 / D
        CB2[32 * p : 32 * p + D, 32 * p] = 1.0 / D
    c["cb1"] = CB1.astype(ml_dtypes.bfloat16)
    c["cb2"] = CB2.astype(ml_dtypes.bfloat16)

    # broadcast selectors: col m -> 1 at row 32*(m//32); selg folds g
    SELR = np.zeros((128, 128), np.float32)
    SELG1 = np.zeros((128, 128), np.float32)
    SELG2 = np.zeros((128, 128), np.float32)
    for m in range(128):
        SELR[32 * (m // 32), m] = 1.0
        if m % 32 < D:
            SELG1[32 * (m // 32), m] = g1[m % 32]
            SELG2[32 * (m // 32), m] = g2[m % 32]
    c["selr"] = SELR.astype(ml_dtypes.bfloat16)
    c["selg1"] = SELG1.astype(ml_dtypes.bfloat16)
    c["selg2"] = SELG2.astype(ml_dtypes.bfloat16)
    c["selb"] = SELR.astype(ml_dtypes.bfloat16)

    # FFN W1 lhsT bf16: variant p: rows 32p+d, col 64p+m = W1[m, d]
    W1E = np.zeros((128, 4 * 64), np.float32)
    for p in range(4):
        W1E[32 * p : 32 * p + D, 64 * p : 64 * p + DFF] = W1.T
    c["w1e"] = W1E.astype(ml_dtypes.bfloat16)

    # FFN W2 lhsT bf16: even variant rows 0:48, odd rows 64:112
    W2E = np.zeros((128, 2 * 32), np.float32)
    W2E[0:DFF, 0:D] = W2.T
    W2E[64 : 64 + DFF, 32 : 32 + D] = W2.T
    c["w2e"] = W2E.astype(ml_dtypes.bfloat16)

    c["idtb"] = np.eye(32, dtype=ml_dtypes.bfloat16)

    # banded biases (only used when nonzero)
    GB = np.zeros((128, 2), np.float32)
    for p in range(4):
        GB[32 * p : 32 * p + D, 0] = b1
        GB[32 * p : 32 * p + D, 1] = b2
    c["gb"] = GB
    return c


CONST_SPECS = {
    "wqk1": ((128, 128), BF16),
    "wk2": ((128, 128), BF16),
    "woe": ((128, 32), BF16),
    "cb1": ((128, 128), BF16),
    "cb2": ((128, 128), BF16),
    "selr": ((128, 128), BF16),
    "selg1": ((128, 128), BF16),
    "selg2": ((128, 128), BF16),
    "selb": ((128, 128), BF16),
    "w1e": ((128, 4 * 64), BF16),
    "w2e": ((128, 2 * 32), BF16),
    "idtb": ((32, 32), BF16),
    "gb": ((128, 2), F32),
}


def _pin_act_tables():
    """Pin Exp/Ln to natural_log_exp_and_others so LN's Ln+Exp never thrash."""
    import concourse.bacc as _bacc
    if getattr(_bacc, "_act_tables_pinned", False):
        return
    _orig = _bacc.get_activation_tables

    def _patched(arch):
        tables = dict(_orig(arch))
        keep = "natural_log_exp_and_others"
        for name in list(tables):
            if name != keep and (AF.Exp in tables[name] or AF.Ln in tables[name]):
                tables[name] = set()
        return tables

    _bacc.get_activation_tables = _patched
    _bacc._act_tables_pinned = True


def build_nc(nb: int = NB, use_b: bool = False) -> bass.Bass:
    _pin_act_tables()
    ngroups = nb // 4
    nc = bacc.Bacc()
    x_in = nc.dram_tensor("x", [nb, S, D], F32, kind="ExternalInput")
    out = nc.dram_tensor("out", [nb, S, D], F32, kind="ExternalOutput")
    cin = {
        k: nc.dram_tensor(k, list(sh), dt, kind="ExternalInput")
        for k, (sh, dt) in CONST_SPECS.items()
    }

    with tile.TileContext(nc) as tc:
        import contextlib

        ctx = contextlib.ExitStack()
        with ctx:
            constp = ctx.enter_context(tc.tile_pool(name="consts", bufs=1))
            vdp = ctx.enter_context(tc.tile_pool(name="vdp", bufs=1))
            xnp = ctx.enter_context(tc.tile_pool(name="xn", bufs=2))
            xtp = ctx.enter_context(tc.tile_pool(name="xt", bufs=2))
            xtbp = ctx.enter_context(tc.tile_pool(name="xtb", bufs=2))
            qkp = ctx.enter_context(tc.tile_pool(name="qk", bufs=2))
            ep = ctx.enter_context(tc.tile_pool(name="e", bufs=2))
            uosp = ctx.enter_context(tc.tile_pool(name="uos", bufs=2))
            rcpp = ctx.enter_context(tc.tile_pool(name="rcp", bufs=2))
            otp = ctx.enter_context(tc.tile_pool(name="ot", bufs=2))
            y1p = ctx.enter_context(tc.tile_pool(name="y1", bufs=2))
            lnp = ctx.enter_context(tc.tile_pool(name="ln", bufs=2))
            x1p = ctx.enter_context(tc.tile_pool(name="x1", bufs=2))
            hsp = ctx.enter_context(tc.tile_pool(name="hs", bufs=2))
            fsp = ctx.enter_context(tc.tile_pool(name="fs", bufs=2))
            ytp = ctx.enter_context(tc.tile_pool(name="yt", bufs=2))
            # PSUM: stt 2x[128,1024] + half 2x[128,512] + uo 2x[128,512] = 8 banks
            psA = ctx.enter_context(tc.tile_pool(name="psA", bufs=2, space="PSUM"))
            psB = ctx.enter_context(tc.tile_pool(name="psB", bufs=2, space="PSUM"))
            psC = ctx.enter_context(tc.tile_pool(name="psC", bufs=2, space="PSUM"))

            # prefetch group 0's x before the const DMAs so batch 0's
            # transpose chain overlaps the constant loads
            U0 = xnp.tile([128, 16, 32], F32, name="xu0")
            nc.gpsimd.memset(U0[:, :, D:32], 0.0)
            for pp in range(4):
                nc.sync.dma_start(
                    out=U0[32 * pp : 32 * pp + 32, :, 0:D],
                    in_=x_in[pp].rearrange("(f c) d -> c f d", c=32),
                )
            C = {}
            for k, (sh, dt) in CONST_SPECS.items():
                t = constp.tile(list(sh), dt, name=f"c_{k}")
                nc.sync.dma_start(out=t, in_=cin[k][:, :])
                C[k] = t
            eps_t = constp.tile([128, 1], F32, name="c_eps")
            nc.vector.memset(eps_t, EPS)

            # two persistent VD tiles [128, t4, hp2, i2, m128] fp8 (ping-pong).
            # ones preset at m = 32*(2hp+i) + {0, 7..31} inside each head band
            # (denominator column + finite padding rows for UO).
            vd_tiles = []
            for v in range(2):
                vt = vdp.tile([128, 4, 2, 2, 128], FP8, name=f"vd{v}")
                nc.gpsimd.memset(vt[:, :, :, :, :], 0.0)
                for hp in range(2):
                    for i in range(2):
                        m0 = 32 * (2 * hp + i)
                        nc.gpsimd.memset(vt[:, :, hp, i, m0 : m0 + 1], 1.0)
                        nc.gpsimd.memset(vt[:, :, hp, i, m0 + 7 : m0 + 32], 1.0)
                vd_tiles.append(vt)

            def ln_stages(Y, selg, bslice, OUT, outb=None):
                """LayerNorm over banded d of Y [128,512] fp32r, split into 4
                stages so PE never blocks on the DVE/ACT hops in between."""
                st = {}

                def s1():
                    mps = psB.tile([128, S], F32, name="mps", tag="half")
                    nc.tensor.matmul(mps[:, :], C["cb1"][:, :], Y[:, :],
                                     start=True, stop=True)
                    MU1 = lnp.tile([128, S], BF16, name="mu1", tag="mu")
                    nc.vector.tensor_copy(MU1[:, :], mps[:, :])
                    st["MU1"] = MU1

                def s2():
                    bcmu = psB.tile([128, S], F32, name="bcmu", tag="half")
                    nc.tensor.matmul(bcmu[:, :], C["selr"][:, :], st["MU1"][:, :],
                                     start=True, stop=True)
                    YC = lnp.tile([128, S], F32, name="yc", tag="yc")
                    nc.vector.tensor_add(YC[:, :], Y[:, :], bcmu[:, :])
                    YC2 = lnp.tile([128, S], BF16, name="yc2", tag="yc2")
                    nc.vector.tensor_mul(YC2[:, :], YC[:, :], YC[:, :])
                    st["YC"], st["YC2"] = YC, YC2

                def s3():
                    vps = psB.tile([128, S], F32, name="vps", tag="half")
                    nc.tensor.matmul(vps[:, :], C["cb2"][:, :], st["YC2"][:, :],
                                     start=True, stop=True)
                    LNV = lnp.tile([128, S], F32, name="lnv", tag="lnv")
                    nc.scalar.activation(LNV[:, :], vps[:, :], AF.Ln,
                                         bias=eps_t[:, :])
                    RS1 = lnp.tile([128, S], BF16, name="rs1", tag="rs")
                    nc.scalar.activation(RS1[:, :], LNV[:, :], AF.Exp, scale=-0.5)
                    st["RS1"] = RS1

                def s4():
                    bcrs = psB.tile([128, S], F32, name="bcrs", tag="half")
                    nc.tensor.matmul(bcrs[:, :], selg[:, :], st["RS1"][:, :],
                                     start=True, stop=True)
                    if use_b:
                        TMP = lnp.tile([128, S], F32, name="lntmp", tag="tmp")
                        nc.vector.tensor_mul(TMP[:, :], st["YC"][:, :], bcrs[:, :])
                        nc.vector.tensor_scalar(out=OUT[:, :], in0=TMP[:, :],
                                                scalar1=bslice, scalar2=None,
                                                op0=ALU.add)
                    else:
                        nc.vector.tensor_mul(OUT[:, :], st["YC"][:, :], bcrs[:, :])
                    if outb is not None:
                        nc.gpsimd.tensor_copy(outb[:, :], OUT[:, :])

                return [s1, s2, s3, s4]

            def qkv_part(g, p, XT4b):
                """QKV projections + V^T extraction for batch 4g+p."""
                ps1 = psB.tile([128, S], F32, name="ps1", tag="half")
                nc.tensor.matmul(
                    ps1[:, :], C["wqk1"][32 * p : 32 * p + D, :],
                    XT4b[32 * p : 32 * p + D, :],
                    start=True, stop=True, tile_position=(32 * p, 0),
                )
                ps2 = psB.tile([128, S], F32, name="ps2", tag="half")
                nc.tensor.matmul(
                    ps2[:, :], C["wk2"][32 * p : 32 * p + D, :],
                    XT4b[32 * p : 32 * p + D, :],
                    start=True, stop=True, tile_position=(32 * p, 0),
                )
                QK = qkp.tile([128, 2 * S], BF16, name="qk")
                nc.vector.tensor_copy(QK[:, 0:S], ps1[:, :])
                nc.scalar.activation(QK[:, S : 2 * S], ps2[:, :], AF.Copy)

                # V^T via PE transposes of rows 0:32 (V lives at rows 8:32)
                psv = psB.tile([128, 4 * 32], BF16, name="psv", tag="half")
                for t in range(4):
                    nc.tensor.transpose(
                        psv[:, 32 * t : 32 * (t + 1)],
                        QK[0:32, 128 * t : 128 * (t + 1)],
                        C["idtb"][:, :],
                    )
                VD = vd_tiles[(4 * g + p) % 2]
                # VD[k, t, hp, i, 32*(2hp+i)+1 : +7] = psv[k, 32t+8+6(2hp+i) : +6]
                vd_dst = bass.AP(
                    tensor=VD.tensor, offset=VD.offset + 1,
                    ap=[list(VD.ap[0]), [512, 4], [320, 2], [160, 2], [1, 6]],
                )
                vd_src = bass.AP(
                    tensor=psv.tensor, offset=psv.offset + 8,
                    ap=[list(psv.ap[0]), [32, 4], [12, 2], [6, 2], [1, 6]],
                )
                nc.vector.tensor_copy(vd_dst, vd_src)
                return {"QK": QK, "VD": VD, "p": p}

            def scores_part(st, filler):
                """scores (S^T, raw) + E-gen; pops one filler per score pair."""
                QK, VD = st["QK"], st["VD"]
                E = ep.tile([128, 4, 2, 2, S], FP8, name="e")
                for t in range(4):
                    for hp in range(2):
                        stt = psA.tile([128, 2 * S], F32, name="stt", tag="big")
                        for i in range(2):
                            h = 2 * hp + i
                            nc.tensor.matmul(
                                stt[:, 512 * i : 512 * (i + 1)],
                                QK[32 * h : 32 * h + HD,
                                   S + 128 * t : S + 128 * (t + 1)],
                                QK[32 * h : 32 * h + HD, 0:S],
                                start=True, stop=True,
                                tile_position=(32 * h, 0),
                                skip_group_check=True,
                            )
                        eng = EGEN_ENGINES[2 * t + hp]
                        e_out = E[:, t, hp, :, :].bitcast(U8)
                        s_in = stt.rearrange("p (i q) -> p i q", i=2)
                        if eng == "act":
                            nc.scalar.activation(e_out, s_in, AF.Copy,
                                                 bias=EB, scale=EA)
                        else:
                            nc.vector.tensor_scalar(
                                out=e_out, in0=s_in, scalar1=EA, scalar2=EB,
                                op0=ALU.mult, op1=ALU.add,
                            )
                        if filler:
                            filler.pop(0)()
                st["E"] = E
                return st

            def av_closures(state):
                """fp8 DoubleRow AV as 8 single-matmul closures (interleavable)."""
                E, VD = state["E"], state["VD"]
                UO = psC.tile([128, S], F32, name="uo", tag="uo")
                state["UO"] = UO

                def mk(t, hp, idx):
                    def go():
                        nc.tensor.matmul(
                            UO[:, :], VD[:, t, hp, :, :], E[:, t, hp, :, :],
                            start=(idx == 0), stop=(idx == 7),
                            perf_mode=DR, tile_position=(0, 0),
                            skip_group_check=True,
                        )
                    return go

                return [mk(t, hp, 2 * t + hp) for t in range(4) for hp in range(2)]

            def tail_closures(state, XT4, Y1):
                """normalize + Wo + residual for a batch, as two closures."""
                p = state["p"]
                hold = {}

                def t1():
                    UOs = uosp.tile([128, S], BF16, name="uos")
                    nc.scalar.activation(UOs[:, :], state["UO"][:, :], AF.Copy)
                    denps = psB.tile([128, S], F32, name="denps", tag="half")
                    nc.tensor.matmul(denps[:, :], C["selb"][:, :], UOs[:, :],
                                     start=True, stop=True)
                    hold["UOs"], hold["denps"] = UOs, denps

                def t2a():
                    RCP = rcpp.tile([128, S], F32, name="rcpt")
                    nc.vector.reciprocal_approx_fast(RCP[:, :], hold["denps"][:, :])
                    OTn = otp.tile([128, S], BF16, name="otn")
                    nc.vector.tensor_mul(OTn[:, :], hold["UOs"][:, :], RCP[:, :])
                    hold["OTn"] = OTn

                def t2b():
                    wops = psB.tile([32, S], F32, name="wops", tag="half")
                    nc.tensor.matmul(wops[:, :], C["woe"][:, :], hold["OTn"][:, :],
                                     start=True, stop=True)
                    nc.vector.tensor_add(
                        Y1[32 * p : 32 * p + 32, :], wops[:, :],
                        XT4[32 * p : 32 * p + 32, :],
                    )

                return [t1, t2a, t2b]

            def group_stages(g, Y1):
                """LN1 + FFN + LN2 + output store for group g as a stage list."""
                X1 = x1p.tile([128, S], BF16, name="x1")
                stages = ln_stages(Y1, C["selg1"], C["gb"][:, 0:1], X1)
                st = {}

                def ffn_a():
                    F4 = psC.tile([128, S], F32, name="f4", tag="uo")
                    hps = psB.tile([128, S], F32, name="hps", tag="half")
                    for j in range(2):
                        nc.tensor.matmul(
                            hps[64 * j : 64 * j + 64, :],
                            C["w1e"][:, 64 * j : 64 * (j + 1)],
                            X1[:, :],
                            start=True, stop=True, tile_position=(0, 64 * j),
                            skip_group_check=True,
                        )
                    HS = hsp.tile([128, S], BF16, name="hs")
                    nc.scalar.activation(HS[:, :], hps[:, :], AF.Relu)
                    st["F4"], st["HS"] = F4, HS

                def ffn_b():
                    F4 = st["F4"]
                    for j in range(2):
                        nc.tensor.matmul(
                            F4[32 * j : 32 * j + 32, :],
                            C["w2e"][:, 32 * j : 32 * (j + 1)],
                            st["HS"][:, :],
                            start=True, stop=True, tile_position=(0, 32 * j),
                            skip_group_check=True,
                        )
                    hps = psB.tile([128, S], F32, name="hps2", tag="half")
                    for j in range(2):
                        nc.tensor.matmul(
                            hps[64 * j : 64 * j + 64, :],
                            C["w1e"][:, 64 * (2 + j) : 64 * (3 + j)],
                            X1[:, :],
                            start=True, stop=True, tile_position=(0, 64 * j),
                            skip_group_check=True,
                        )
                    HS2 = hsp.tile([128, S], BF16, name="hs2", tag="hs2")
                    nc.scalar.activation(HS2[:, :], hps[:, :], AF.Relu)
                    st["HS2"] = HS2

                def ffn_c():
                    F4 = st["F4"]
                    for j in range(2):
                        nc.tensor.matmul(
                            F4[32 * (2 + j) : 32 * (3 + j), :],
                            C["w2e"][:, 32 * j : 32 * (j + 1)],
                            st["HS2"][:, :],
                            start=True, stop=True, tile_position=(0, 32 * (2 + j)),
                            skip_group_check=True,
                        )
                    FS0 = fsp.tile([128, S], BF16, name="fs0", tag="fs0")
                    nc.scalar.activation(FS0[:, :], F4[:, :], AF.Relu)
                    FS = fsp.tile([128, S], BF16, name="fst", tag="fst")
                    nc.gpsimd.tensor_add(FS[:, :], FS0[:, :], X1[:, :])
                    st["FS"] = FS

                stages += [ffn_a, ffn_b, ffn_c]

                Y2N = ytp.tile([128, S], F32, name="y2n", tag="y2n")
                ln2 = [None]

                def ln2_s1():
                    ln2[0] = ln_stages(st["FS"], C["selg2"], C["gb"][:, 1:2], Y2N)
                    ln2[0][0]()

                def emit_out():
                    Y2T = ytp.tile([128, S], F32, name="y2t", tag="y2t")
                    nc.vector.transpose(Y2T[:, :], Y2N[:, :])
                    for p in range(4):
                        b = 4 * g + p
                        nc.sync.dma_start(
                            out=out[b].rearrange("(f r) d -> r f d", r=32),
                            in_=Y2T[32 * p : 32 * p + 32, :].rearrange(
                                "r (f c) -> r f c", c=32
                            )[:, :, 0:D],
                        )

                stages += [ln2_s1,
                           lambda: ln2[0][1](),
                           lambda: ln2[0][2](),
                           lambda: ln2[0][3](),
                           emit_out]
                return stages

            # slot machine: headA(i) | AV(i-1) | tail(i-2) | <=3 group stages
            states = {}
            heads = {}
            gctx = {}
            pending = []
            nslots = 4 * ngroups

            def emit_slot(i):
                g, p = divmod(i, 4)
                if p == 0:
                    if g == 0:
                        U = U0
                    else:
                        U = xnp.tile([128, 16, 32], F32, name="xu")
                        nc.gpsimd.memset(U[:, :, D:32], 0.0)
                        for pp in range(4):
                            nc.sync.dma_start(
                                out=U[32 * pp : 32 * pp + 32, :, 0:D],
                                in_=x_in[4 * g + pp].rearrange("(f c) d -> c f d", c=32),
                            )
                    XT4 = xtp.tile([128, S], F32, name="xt4")
                    nc.vector.transpose(XT4[:, :], U.rearrange("p a b -> p (a b)"))
                    XT4b = xtbp.tile([128, S], BF16, name="xt4b")
                    nc.vector.tensor_copy(XT4b[:, :], XT4[:, :])
                    Y1 = y1p.tile([128, S], BF16, name="y1")
                    gctx[g] = (XT4, XT4b, Y1)
                XT4, XT4b, Y1 = gctx[g]
                base = []
                post = []
                gpush = None
                if i - 1 in states:
                    base += av_closures(states[i - 1])
                if i - 2 in states:
                    s = states.pop(i - 2)
                    g2 = (i - 2) // 4
                    tcs = tail_closures(s, gctx[g2][0], gctx[g2][2])
                    base += tcs[:2]
                    post.append(tcs[2])
                    if (i - 2) % 4 == 3:
                        gpush = g2   # push AFTER t2b writes Y1's last band
                gs = []
                for _ in range(3):
                    if pending:
                        gs.append(pending.pop(0))
                # weave group stages ~4 filler positions apart so their
                # dependency chains clear before the next stage's PE matmul
                filler = (base[0:3] + gs[0:1] + base[3:6] + gs[1:2]
                          + base[6:9] + gs[2:3] + base[9:])
                if i not in heads:
                    heads[i] = qkv_part(g, p, XT4b)
                if i == 0:
                    def _hoist1():
                        heads[1] = qkv_part(g, 1, XT4b)
                    filler.insert(2, _hoist1)
                states[i] = scores_part(heads.pop(i), filler)
                while filler:
                    filler.pop(0)()
                for f in post:
                    f()
                if gpush is not None:
                    pending.extend(group_stages(gpush, gctx[gpush][2]))

            for i in range(nslots):
                emit_slot(i)
            # drain: interleave the leftover stages of the second-to-last
            # group with the last group's stages so one group's matmuls fill
            # the other's ACT/DVE chain latency
            for f in av_closures(states[nslots - 1]):
                f()
            for i in (nslots - 2, nslots - 1):
                s = states.pop(i)
                g2 = i // 4
                for f in tail_closures(s, gctx[g2][0], gctx[g2][2]):
                    f()
            pre = list(pending)
            last = group_stages(ngroups - 1, gctx[ngroups - 1][2])
            pending.clear()
            while pre or last:
                if last:
                    last.pop(0)()
                if pre:
                    pre.pop(0)()

    nc.compile()
    return nc


def build_nc(nb: int = NB, use_b: bool = False) -> bass.Bass:
    _pin_act_tables()
    ngroups = nb // 4
    nc = bacc.Bacc()
    x_in = nc.dram_tensor("x", [nb, S, D], F32, kind="ExternalInput")
    out = nc.dram_tensor("out", [nb, S, D], F32, kind="ExternalOutput")
    cin = {
        k: nc.dram_tensor(k, list(sh), dt, kind="ExternalInput")
        for k, (sh, dt) in CONST_SPECS.items()
    }

    with tile.TileContext(nc) as tc:
        import contextlib

        ctx = contextlib.ExitStack()
        with ctx:
            constp = ctx.enter_context(tc.tile_pool(name="consts", bufs=1))
            vdp = ctx.enter_context(tc.tile_pool(name="vdp", bufs=1))
            xnp = ctx.enter_context(tc.tile_pool(name="xn", bufs=2))
            xtp = ctx.enter_context(tc.tile_pool(name="xt", bufs=2))
            xtbp = ctx.enter_context(tc.tile_pool(name="xtb", bufs=2))
            qkp = ctx.enter_context(tc.tile_pool(name="qk", bufs=2))
            ep = ctx.enter_context(tc.tile_pool(name="e", bufs=2))
            uosp = ctx.enter_context(tc.tile_pool(name="uos", bufs=2))
            rcpp = ctx.enter_context(tc.tile_pool(name="rcp", bufs=2))
            otp = ctx.enter_context(tc.tile_pool(name="ot", bufs=2))
            y1p = ctx.enter_context(tc.tile_pool(name="y1", bufs=2))
            lnp = ctx.enter_context(tc.tile_pool(name="ln", bufs=2))
            x1p = ctx.enter_context(tc.tile_pool(name="x1", bufs=2))
            hsp = ctx.enter_context(tc.tile_pool(name="hs", bufs=2))
            fsp = ctx.enter_context(tc.tile_pool(name="fs", bufs=2))
            ytp = ctx.enter_context(tc.tile_pool(name="yt", bufs=2))
            # PSUM: stt 2x[128,1024] + half 2x[128,512] + uo 2x[128,512] = 8 banks
            psA = ctx.enter_context(tc.tile_pool(name="psA", bufs=2, space="PSUM"))
            psB = ctx.enter_context(tc.tile_pool(name="psB", bufs=2, space="PSUM"))
            psC = ctx.enter_context(tc.tile_pool(name="psC", bufs=2, space="PSUM"))

            # prefetch group 0's x before the const DMAs so batch 0's
            # transpose chain overlaps the constant loads
            U0 = xnp.tile([128, 16, 32], F32, name="xu0")
            nc.gpsimd.memset(U0[:, :, D:32], 0.0)
            for pp in range(4):
                nc.sync.dma_start(
                    out=U0[32 * pp : 32 * pp + 32, :, 0:D],
                    in_=x_in[pp].rearrange("(f c) d -> c f d", c=32),
                )
            C = {}
            for k, (sh, dt) in CONST_SPECS.items():
                t = constp.tile(list(sh), dt, name=f"c_{k}")
                nc.sync.dma_start(out=t, in_=cin[k][:, :])
                C[k] = t
            eps_t = constp.tile([128, 1], F32, name="c_eps")
            nc.vector.memset(eps_t, EPS)

            # two persistent VD tiles [128, t4, hp2, i2, m128] fp8 (ping-pong).
            # ones preset at m = 32*(2hp+i) + {0, 7..31} inside each head band
            # (denominator column + finite padding rows for UO).
            vd_tiles = []
            for v in range(2):
                vt = vdp.tile([128, 4, 2, 2, 128], FP8, name=f"vd{v}")
                nc.gpsimd.memset(vt[:, :, :, :, :], 0.0)
                for hp in range(2):
                    for i in range(2):
                        m0 = 32 * (2 * hp + i)
                        nc.gpsimd.memset(vt[:, :, hp, i, m0 : m0 + 1], 1.0)
                        nc.gpsimd.memset(vt[:, :, hp, i, m0 + 7 : m0 + 32], 1.0)
                vd_tiles.append(vt)

            def ln_stages(Y, selg, bslice, OUT, outb=None):
                """LayerNorm over banded d of Y [128,512] fp32r, split into 4
                stages so PE never blocks on the DVE/ACT hops in between."""
                st = {}

                def s1():
                    mps = psB.tile([128, S], F32, name="mps", tag="half")
                    nc.tensor.matmul(mps[:, :], C["cb1"][:, :], Y[:, :],
                                     start=True, stop=True)
                    MU1 = lnp.tile([128, S], BF16, name="mu1", tag="mu")
                    nc.vector.tensor_copy(MU1[:, :], mps[:, :])
                    st["MU1"] = MU1

                def s2():
                    bcmu = psB.tile([128, S], F32, name="bcmu", tag="half")
                    nc.tensor.matmul(bcmu[:, :], C["selr"][:, :], st["MU1"][:, :],
                                     start=True, stop=True)
                    YC = lnp.tile([128, S], F32, name="yc", tag="yc")
                    nc.vector.tensor_add(YC[:, :], Y[:, :], bcmu[:, :])
                    YC2 = lnp.tile([128, S], BF16, name="yc2", tag="yc2")
                    nc.vector.tensor_mul(YC2[:, :], YC[:, :], YC[:, :])
                    st["YC"], st["YC2"] = YC, YC2

                def s3():
                    vps = psB.tile([128, S], F32, name="vps", tag="half")
                    nc.tensor.matmul(vps[:, :], C["cb2"][:, :], st["YC2"][:, :],
                                     start=True, stop=True)
                    LNV = lnp.tile([128, S], F32, name="lnv", tag="lnv")
                    nc.scalar.activation(LNV[:, :], vps[:, :], AF.Ln,
                                         bias=eps_t[:, :])
                    RS1 = lnp.tile([128, S], BF16, name="rs1", tag="rs")
                    nc.scalar.activation(RS1[:, :], LNV[:, :], AF.Exp, scale=-0.5)
                    st["RS1"] = RS1

                def s4():
                    bcrs = psB.tile([128, S], F32, name="bcrs", tag="half")
                    nc.tensor.matmul(bcrs[:, :], selg[:, :], st["RS1"][:, :],
                                     start=True, stop=True)
                    if use_b:
                        TMP = lnp.tile([128, S], F32, name="lntmp", tag="tmp")
                        nc.vector.tensor_mul(TMP[:, :], st["YC"][:, :], bcrs[:, :])
                        nc.vector.tensor_scalar(out=OUT[:, :], in0=TMP[:, :],
                                                scalar1=bslice, scalar2=None,
                                                op0=ALU.add)
                    else:
                        nc.vector.tensor_mul(OUT[:, :], st["YC"][:, :], bcrs[:, :])
                    if outb is not None:
                        nc.gpsimd.tensor_copy(outb[:, :], OUT[:, :])

                return [s1, s2, s3, s4]

            def qkv_part(g, p, XT4b):
                """QKV projections + V^T extraction for batch 4g+p."""
                ps1 = psB.tile([128, S], F32, name="ps1", tag="half")
                nc.tensor.matmul(
                    ps1[:, :], C["wqk1"][32 * p : 32 * p + D, :],
                    XT4b[32 * p : 32 * p + D, :],
                    start=True, stop=True, tile_position=(32 * p, 0),
                )
                ps2 = psB.tile([128, S], F32, name="ps2", tag="half")
                nc.tensor.matmul(
                    ps2[:, :], C["wk2"][32 * p : 32 * p + D, :],
                    XT4b[32 * p : 32 * p + D, :],
                    start=True, stop=True, tile_position=(32 * p, 0),
                )
                QK = qkp.tile([128, 2 * S], BF16, name="qk")
                nc.vector.tensor_copy(QK[:, 0:S], ps1[:, :])
                nc.scalar.activation(QK[:, S : 2 * S], ps2[:, :], AF.Copy)

                # V^T via PE transposes of rows 0:32 (V lives at rows 8:32)
                psv = psB.tile([128, 4 * 32], BF16, name="psv", tag="half")
                for t in range(4):
                    nc.tensor.transpose(
                        psv[:, 32 * t : 32 * (t + 1)],
                        QK[0:32, 128 * t : 128 * (t + 1)],
                        C["idtb"][:, :],
                    )
                VD = vd_tiles[(4 * g + p) % 2]
                # VD[k, t, hp, i, 32*(2hp+i)+1 : +7] = psv[k, 32t+8+6(2hp+i) : +6]
                vd_dst = bass.AP(
                    tensor=VD.tensor, offset=VD.offset + 1,
                    ap=[list(VD.ap[0]), [512, 4], [320, 2], [160, 2], [1, 6]],
                )
                vd_src = bass.AP(
                    tensor=psv.tensor, offset=psv.offset + 8,
                    ap=[list(psv.ap[0]), [32, 4], [12, 2], [6, 2], [1, 6]],
                )
                nc.vector.tensor_copy(vd_dst, vd_src)
                return {"QK": QK, "VD": VD, "p": p}

            def scores_part(st, filler):
                """scores (S^T, raw) + E-gen; pops one filler per score pair."""
                QK, VD = st["QK"], st["VD"]
                E = ep.tile([128, 4, 2, 2, S], FP8, name="e")
                for t in range(4):
                    for hp in range(2):
                        stt = psA.tile([128, 2 * S], F32, name="stt", tag="big")
                        for i in range(2):
                            h = 2 * hp + i
                            nc.tensor.matmul(
                                stt[:, 512 * i : 512 * (i + 1)],
                                QK[32 * h : 32 * h + HD,
                                   S + 128 * t : S + 128 * (t + 1)],
                                QK[32 * h : 32 * h + HD, 0:S],
                                start=True, stop=True,
                                tile_position=(32 * h, 0),
                                skip_group_check=True,
                            )
                        eng = EGEN_ENGINES[2 * t + hp]
                        e_out = E[:, t, hp, :, :].bitcast(U8)
                        s_in = stt.rearrange("p (i q) -> p i q", i=2)
                        if eng == "act":
                            nc.scalar.activation(e_out, s_in, AF.Copy,
                                                 bias=EB, scale=EA)
                        else:
                            nc.vector.tensor_scalar(
                                out=e_out, in0=s_in, scalar1=EA, scalar2=EB,
                                op0=ALU.mult, op1=ALU.add,
                            )
                        if filler:
                            filler.pop(0)()
                st["E"] = E
                return st

            def av_closures(state):
                """fp8 DoubleRow AV as 8 single-matmul closures (interleavable)."""
                E, VD = state["E"], state["VD"]
                UO = psC.tile([128, S], F32, name="uo", tag="uo")
                state["UO"] = UO

                def mk(t, hp, idx):
                    def go():
                        nc.tensor.matmul(
                            UO[:, :], VD[:, t, hp, :, :], E[:, t, hp, :, :],
                            start=(idx == 0), stop=(idx == 7),
                            perf_mode=DR, tile_position=(0, 0),
                            skip_group_check=True,
                        )
                    return go

                return [mk(t, hp, 2 * t + hp) for t in range(4) for hp in range(2)]

            def tail_closures(state, XT4, Y1):
                """normalize + Wo + residual for a batch, as two closures."""
                p = state["p"]
                hold = {}

                def t1():
                    UOs = uosp.tile([128, S], BF16, name="uos")
                    nc.scalar.activation(UOs[:, :], state["UO"][:, :], AF.Copy)
                    denps = psB.tile([128, S], F32, name="denps", tag="half")
                    nc.tensor.matmul(denps[:, :], C["selb"][:, :], UOs[:, :],
                                     start=True, stop=True)
                    hold["UOs"], hold["denps"] = UOs, denps

                def t2a():
                    RCP = rcpp.tile([128, S], F32, name="rcpt")
                    nc.vector.reciprocal_approx_fast(RCP[:, :], hold["denps"][:, :])
                    OTn = otp.tile([128, S], BF16, name="otn")
                    nc.vector.tensor_mul(OTn[:, :], hold["UOs"][:, :], RCP[:, :])
                    hold["OTn"] = OTn

                def t2b():
                    wops = psB.tile([32, S], F32, name="wops", tag="half")
                    nc.tensor.matmul(wops[:, :], C["woe"][:, :], hold["OTn"][:, :],
                                     start=True, stop=True)
                    nc.vector.tensor_add(
                        Y1[32 * p : 32 * p + 32, :], wops[:, :],
                        XT4[32 * p : 32 * p + 32, :],
                    )

                return [t1, t2a, t2b]

            def group_stages(g, Y1):
                """LN1 + FFN + LN2 + output store for group g as a stage list."""
                X1 = x1p.tile([128, S], BF16, name="x1")
                stages = ln_stages(Y1, C["selg1"], C["gb"][:, 0:1], X1)
                st = {}

                def ffn_a():
                    F4 = psC.tile([128, S], F32, name="f4", tag="uo")
                    hps = psB.tile([128, S], F32, name="hps", tag="half")
                    for j in range(2):
                        nc.tensor.matmul(
                            hps[64 * j : 64 * j + 64, :],
                            C["w1e"][:, 64 * j : 64 * (j + 1)],
                            X1[:, :],
                            start=True, stop=True, tile_position=(0, 64 * j),
                            skip_group_check=True,
                        )
                    HS = hsp.tile([128, S], BF16, name="hs")
                    nc.scalar.activation(HS[:, :], hps[:, :], AF.Relu)
                    st["F4"], st["HS"] = F4, HS

                def ffn_b():
                    F4 = st["F4"]
                    for j in range(2):
                        nc.tensor.matmul(
                            F4[32 * j : 32 * j + 32, :],
                            C["w2e"][:, 32 * j : 32 * (j + 1)],
                            st["HS"][:, :],
                            start=True, stop=True, tile_position=(0, 32 * j),
                            skip_group_check=True,
                        )
                    hps = psB.tile([128, S], F32, name="hps2", tag="half")
                    for j in range(2):
                        nc.tensor.matmul(
                            hps[64 * j : 64 * j + 64, :],
                            C["w1e"][:, 64 * (2 + j) : 64 * (3 + j)],
                            X1[:, :],
                            start=True, stop=True, tile_position=(0, 64 * j),
                            skip_group_check=True,
                        )
                    HS2 = hsp.tile([128, S], BF16, name="hs2", tag="hs2")
                    nc.scalar.activation(HS2[:, :], hps[:, :], AF.Relu)
                    st["HS2"] = HS2

                def ffn_c():
                    F4 = st["F4"]
                    for j in range(2):
                        nc.tensor.matmul(
                            F4[32 * (2 + j) : 32 * (3 + j), :],
                            C["w2e"][:, 32 * j : 32 * (j + 1)],
                            st["HS2"][:, :],
                            start=True, stop=True, tile_position=(0, 32 * (2 + j)),
                            skip_group_check=True,
                        )
                    FS0 = fsp.tile([128, S], BF16, name="fs0", tag="fs0")
                    nc.scalar.activation(FS0[:, :], F4[:, :], AF.Relu)
                    FS = fsp.tile([128, S], BF16, name="fst", tag="fst")
                    nc.gpsimd.tensor_add(FS[:, :], FS0[:, :], X1[:, :])
                    st["FS"] = FS

                stages += [ffn_a, ffn_b, ffn_c]

                Y2N = ytp.tile([128, S], F32, name="y2n", tag="y2n")
                ln2 = [None]

                def ln2_s1():
                    ln2[0] = ln_stages(st["FS"], C["selg2"], C["gb"][:, 1:2], Y2N)
                    ln2[0][0]()

                def emit_out():
                    Y2T = ytp.tile([128, S], F32, name="y2t", tag="y2t")
                    nc.vector.transpose(Y2T[:, :], Y2N[:, :])
                    for p in range(4):
                        b = 4 * g + p
                        nc.sync.dma_start(
                            out=out[b].rearrange("(f r) d -> r f d", r=32),
                            in_=Y2T[32 * p : 32 * p + 32, :].rearrange(
                                "r (f c) -> r f c", c=32
                            )[:, :, 0:D],
                        )

                stages += [ln2_s1,
                           lambda: ln2[0][1](),
                           lambda: ln2[0][2](),
                           lambda: ln2[0][3](),
                           emit_out]
                return stages

            # slot machine: headA(i) | AV(i-1) | tail(i-2) | <=3 group stages
            states = {}
            heads = {}
            gctx = {}
            pending = []
            nslots = 4 * ngroups

            def emit_slot(i):
                g, p = divmod(i, 4)
                if p == 0:
                    if g == 0:
                        U = U0
                    else:
                        U = xnp.tile([128, 16, 32], F32, name="xu")
                        nc.gpsimd.memset(U[:, :, D:32], 0.0)
                        for pp in range(4):
                            nc.sync.dma_start(
                                out=U[32 * pp : 32 * pp + 32, :, 0:D],
                                in_=x_in[4 * g + pp].rearrange("(f c) d -> c f d", c=32),
                            )
                    XT4 = xtp.tile([128, S], F32, name="xt4")
                    nc.vector.transpose(XT4[:, :], U.rearrange("p a b -> p (a b)"))
                    XT4b = xtbp.tile([128, S], BF16, name="xt4b")
                    nc.vector.tensor_copy(XT4b[:, :], XT4[:, :])
                    Y1 = y1p.tile([128, S], BF16, name="y1")
                    gctx[g] = (XT4, XT4b, Y1)
                XT4, XT4b, Y1 = gctx[g]
                base = []
                post = []
                gpush = None
                if i - 1 in states:
                    base += av_closures(states[i - 1])
                if i - 2 in states:
                    s = states.pop(i - 2)
                    g2 = (i - 2) // 4
                    tcs = tail_closures(s, gctx[g2][0], gctx[g2][2])
                    base += tcs[:2]
                    post.append(tcs[2])
                    if (i - 2) % 4 == 3:
                        gpush = g2   # push AFTER t2b writes Y1's last band
                gs = []
                for _ in range(3):
                    if pending:
                        gs.append(pending.pop(0))
                # weave group stages ~4 filler positions apart so their
                # dependency chains clear before the next stage's PE matmul
                filler = (base[0:3] + gs[0:1] + base[3:6] + gs[1:2]
                          + base[6:9] + gs[2:3] + base[9:])
                if i not in heads:
                    heads[i] = qkv_part(g, p, XT4b)
                if i == 0:
                    def _hoist1():
                        heads[1] = qkv_part(g, 1, XT4b)
                    filler.insert(2, _hoist1)
                states[i] = scores_part(heads.pop(i), filler)
                while filler:
                    filler.pop(0)()
                for f in post:
                    f()
                if gpush is not None:
                    pending.extend(group_stages(gpush, gctx[gpush][2]))

            for i in range(nslots):
                emit_slot(i)
            # drain: interleave the leftover stages of the second-to-last
            # group with the last group's stages so one group's matmuls fill
            # the other's ACT/DVE chain latency
            for f in av_closures(states[nslots - 1]):
                f()
            for i in (nslots - 2, nslots - 1):
                s = states.pop(i)
                g2 = i // 4
                for f in tail_closures(s, gctx[g2][0], gctx[g2][2]):
                    f()
            pre = list(pending)
            last = group_stages(ngroups - 1, gctx[ngroups - 1][2])
            pending.clear()
            while pre or last:
                if last:
                    last.pop(0)()
                if pre:
                    pre.pop(0)()

    nc.compile()
    return nc


def _enable_ldw_opt():
    """Flip walrus --enable-ldw-opt to true (hides LDWEIGHTS behind matmuls)."""
    import concourse.bass_utils as _bu
    if getattr(_bu, "_ldw_opt_patched", False):
        return
    _orig = _bu.run_command

    def _patched(cmd, *a, **kw):
        if isinstance(cmd, list):
            cmd = ["--enable-ldw-opt=true" if c == "--enable-ldw-opt=false" else c
                   for c in cmd]
        return _orig(cmd, *a, **kw)

    _bu.run_command = _patched
    _bu._ldw_opt_patched = True


_NC_CACHE: dict[tuple, bass.Bass] = {}


def _get_nc(nb: int, use_b: bool = False) -> bass.Bass:
    key = (nb, use_b)
    if key not in _NC_CACHE:
        _NC_CACHE[key] = build_nc(nb, use_b)
    return _NC_CACHE[key]


def kernel(x, Wq, Wk, Wv, Wo, W1, W2, g1, b1, g2, b2):
    x = np.asarray(x, np.float32)
    args = [np.asarray(a, np.float32) for a in (Wq, Wk, Wv, Wo, W1, W2, g1, b1, g2, b2)]
    consts = _host_consts(*args)
    use_b = bool(np.any(args[7]) or np.any(args[9]))
    nc = _get_nc(NB, use_b)
    in_maps = []
    for c in range(NCORES):
        m = {"x": np.ascontiguousarray(x[c * NB : (c + 1) * NB])}
        m.update(consts)
        in_maps.append(m)
    res = run_bass_kernel_spmd(nc, in_maps, list(range(NCORES)))
    return np.concatenate([r["out"] for r in res.results], axis=0)
